# revision 1
# baseline (speedup 1.0000x reference)
"""DiT block kernel for Trainium2 (Bass/Tile), 8-core data-parallel.

Strategy:
  - Pure data parallelism: batch B=8, one batch element per NeuronCore, no
    collectives.
  - Activations kept channel-major (x^T [C, N]) on-chip so every GEMM has its
    contraction dim on the partition axis; weights are PE-transposed on the fly.
  - All GEMMs in float32r (TF32-rate, fp32 storage) with 512/256 moving dim.
  - Causal attention computed as S^T = K^T-tiles @ Q^T with softmax row-sums
    obtained by augmenting V with a ones column; fully-masked tiles skipped,
    diagonal-straddling tiles masked post-exp.
"""

import sys

sys.path.insert(0, "/opt/trn_rl_repo")

import numpy as np

import concourse.bass as bass
import concourse.bacc as bacc
import concourse.mybir as mybir
from concourse import library_config
from concourse.tile import TileContext

F32 = mybir.dt.float32
F32R = mybir.dt.float32r
BF16 = mybir.dt.bfloat16
AF = mybir.ActivationFunctionType
OP = mybir.AluOpType


def r(ap):
    return ap.bitcast(F32R)


def build_program(N=1024, C=1024, H=16, DFF=4096, head_group=8, dff_group=4,
                  n_cores=8, sim_safe=False, stop_after=None, loop_iters=None):
    D = 64
    NT, CT, DT = N // 128, C // 128, DFF // 128
    NJ = N // 512
    HG = head_group
    NHG = H // HG
    G = dff_group
    NG = DT // G
    PK = min(4, CT)            # transpose pack (blocks per psum tile)
    assert H % HG == 0 and DT % G == 0 and N % 512 == 0 and CT % PK == 0
    assert HG % 2 == 0 and D == 64

    nc = bacc.Bacc("TRN2", target_bir_lowering=False, debug=False,
                   num_devices=n_cores, num_swdge_queues=4)

    x_d = nc.dram_tensor("x", [N, C], F32, kind="ExternalInput")
    c_d = nc.dram_tensor("cvec", [C], F32, kind="ExternalInput")
    qkvw_d = nc.dram_tensor("qkv_w", [3 * C, C], F32, kind="ExternalInput")
    qkvb_d = nc.dram_tensor("qkv_b", [3 * C], F32, kind="ExternalInput")
    projw_d = nc.dram_tensor("proj_w", [C, C], F32, kind="ExternalInput")
    projb_d = nc.dram_tensor("proj_b", [C], F32, kind="ExternalInput")
    fc1w_d = nc.dram_tensor("fc1_w", [DFF, C], F32, kind="ExternalInput")
    fc1b_d = nc.dram_tensor("fc1_b", [DFF], F32, kind="ExternalInput")
    fc2w_d = nc.dram_tensor("fc2_w", [C, DFF], F32, kind="ExternalInput")
    fc2b_d = nc.dram_tensor("fc2_b", [C], F32, kind="ExternalInput")
    adaw_d = nc.dram_tensor("ada_w", [6 * C, C], F32, kind="ExternalInput")
    adab_d = nc.dram_tensor("ada_b", [6 * C], F32, kind="ExternalInput")
    out_d = nc.dram_tensor("out", [N, C], F32, kind="ExternalOutput")

    from contextlib import ExitStack
    with TileContext(nc) as tc, ExitStack() as ctx:
        consts = ctx.enter_context(tc.tile_pool(name="consts", bufs=1))
        sb = ctx.enter_context(tc.tile_pool(name="sb", bufs=1))
        wnat_p = ctx.enter_context(tc.tile_pool(name="wnat", bufs=3))
        ada_p = ctx.enter_context(tc.tile_pool(name="adap", bufs=2))
        wt_p = ctx.enter_context(tc.tile_pool(name="wt", bufs=2))
        wtv_p = ctx.enter_context(tc.tile_pool(name="wtv", bufs=1))
        pt_p = ctx.enter_context(tc.tile_pool(name="pt", bufs=3))
        rows_p = ctx.enter_context(tc.tile_pool(name="rows", bufs=3))
        bc_p = ctx.enter_context(tc.tile_pool(name="bc", bufs=2))
        sq_p = ctx.enter_context(tc.tile_pool(name="sqp", bufs=3))

        ps_tp = ctx.enter_context(
            tc.tile_pool(name="ps_tp", bufs=2, space="PSUM"))
        ps_mm = ctx.enter_context(
            tc.tile_pool(name="ps_mm", bufs=2, space="PSUM"))
        ps_row = ctx.enter_context(
            tc.tile_pool(name="ps_row", bufs=2, space="PSUM"))
        ps_o = ctx.enter_context(
            tc.tile_pool(name="ps_o", bufs=2, space="PSUM"))

        from contextlib import nullcontext
        loop_cm = tc.For_i(0, loop_iters, 1) if loop_iters else nullcontext()
        with loop_cm:
            _dmac = [0]

            def dma_rr(out, in_):
                i = _dmac[0]; _dmac[0] += 1
                eng = (nc.sync, nc.scalar, nc.gpsimd)[i % 3]
                eng.dma_start(out=out, in_=in_)
            ident = consts.tile([128, 128], F32, tag="ident")
            nc.gpsimd.memset(ident, 0.0)
            nc.gpsimd.affine_select(
                out=ident, in_=ident, compare_op=OP.not_equal, fill=1.0,
                base=0, pattern=[[-1, 128]], channel_multiplier=1)
            # ================= standing tensors =================
            xres = sb.tile([128, CT, N], F32, tag="xres")

            for ni in range(NT):
                natx = wnat_p.tile([128, C], F32, tag="wnat")
                dma_rr(natx, x_d[ni * 128:(ni + 1) * 128, :])
                for cp in range(CT // PK):
                    pst = ps_tp.tile([128, 128 * PK], F32, tag="tp")
                    for k in range(PK):
                        ci = cp * PK + k
                        nc.tensor.matmul(pst[:, k * 128:(k + 1) * 128],
                                         natx[:, ci * 128:(ci + 1) * 128], ident,
                                         is_transpose=True, start=True, stop=True)
                    nc.vector.tensor_copy(
                        xres[:, cp * PK:(cp + 1) * PK, ni * 128:(ni + 1) * 128],
                        pst.rearrange("p (a b) -> p a b", a=PK))

            def _stop(tag_):
                return stop_after is not None and stop_after == tag_

            # ================= constants =================
            # ================= constants =================
            ident_r = consts.tile([128, 128], F32R, tag="identr")
            nc.vector.tensor_copy(ident_r, ident)
            ones65f = wnat_p.tile([65, 128], F32, tag="bnat")
            nc.vector.memset(ones65f, 1.0)
            ones65 = consts.tile([65, 128], F32R, tag="ones65")
            nc.vector.tensor_copy(ones65, ones65f)

            masks = consts.tile([128, 4, 512], BF16, tag="masks")
            nc.gpsimd.memset(masks, 1.0)
            for i in range(4):
                # keep where n >= m  <=>  s - r - delta >= 0 (delta = 128i)
                nc.gpsimd.affine_select(
                    out=masks[:, i, :], in_=masks[:, i, :], compare_op=OP.is_ge,
                    fill=0.0, base=-(128 * i), pattern=[[1, 512]],
                    channel_multiplier=-1)

            ones_invCf = consts.tile([128, 1], F32, tag="onescf")
            nc.gpsimd.memset(ones_invCf, 1.0 / C)
            ones_invC = consts.tile([128, 1], F32R, tag="onesc")
            nc.vector.tensor_copy(ones_invC, ones_invCf)
            eps_t = consts.tile([1, 1], F32, tag="eps")
            nc.vector.memset(eps_t, 1e-6)

            def bias_T(src, nch, tag):
                t = consts.tile([128, nch], F32, tag=tag)
                natb = wnat_p.tile([nch, 128], F32, tag="bnat")
                nc.sync.dma_start(out=natb,
                                  in_=src.rearrange("(a b) -> a b", b=128))
                pst = ps_tp.tile([128, 128 * PK], F32, tag="tp")
                nc.tensor.matmul(pst[:, 0:nch], natb, ident[0:nch, 0:nch],
                                 is_transpose=True, start=True, stop=True)
                nc.vector.tensor_copy(t, pst[:, 0:nch])
                return t

            qkvb_t = bias_T(qkvb_d.ap(), 3 * CT, "qkvbt")
            projb_t = bias_T(projb_d.ap(), CT, "projbt")
            fc1b_t = bias_T(fc1b_d.ap(), DT, "fc1bt")
            fc2b_t = bias_T(fc2b_d.ap(), CT, "fc2bt")
            adab_t = bias_T(adab_d.ap(), 6 * CT, "adabt")

            # ---- adaLN: silu(c) broadcast ----
            crow = wnat_p.tile([1, C], F32, tag="wnat")
            nc.sync.dma_start(out=crow, in_=c_d.ap().rearrange("(a c) -> a c", a=1))
            silu_row = wnat_p.tile([1, C], F32R, tag="wnat")
            nc.scalar.activation(silu_row, crow, AF.Sigmoid)
            nc.vector.tensor_mul(silu_row, silu_row, crow)
            silu_b = sb.tile([128, C], F32, tag="oT")
            for w0 in range(0, C, 512):
                w = min(512, C - w0)
                pb = ps_row.tile([128, 512], F32, tag="row")
                nc.tensor.matmul(pb[:, 0:w], ones65[0:1, :],
                                 silu_row[0:1, w0:w0 + w], start=True, stop=True)
                nc.vector.tensor_copy(silu_b[:, w0:w0 + w], pb[:, 0:w])

            modsb = consts.tile([128, 6 * CT], F32, tag="modsb")
            def ada_part(jts):
                adadump = wnat_p.tile([128, C], F32, tag="wnat")
                for jt in jts:
                    anat = ada_p.tile([128, C], F32, tag="ada")
                    dma_rr(anat, adaw_d[jt * 128:(jt + 1) * 128, :])
                    nc.gpsimd.tensor_mul(anat, anat, silu_b)
                    nc.scalar.activation(adadump, anat, AF.Identity,
                                         accum_out=modsb[:, jt:jt + 1])
                lo, hi = min(jts), max(jts) + 1
                nc.vector.tensor_add(modsb[:, lo:hi], modsb[:, lo:hi],
                                     adab_t[:, lo:hi])

            # shift/scale_msa now (LN1 path); rest after attention is launched
            ada_part(range(0, 2 * CT))
            sp_msa = consts.tile([128, CT], F32, tag="spmsa")
            nc.vector.tensor_scalar(sp_msa, modsb[:, CT:2 * CT], 1.0, None, OP.add)

            # ================= helpers =================
            def layer_norm(dst, sh_off, sp_tile):
                for nj in range(NJ):
                    njs = slice(nj * 512, (nj + 1) * 512)
                    s_mu = ps_row.tile([1, 512], F32, tag="row")
                    for ci in range(CT):
                        xrr = sq_p.tile([128, 512], F32R, tag="sq")
                        nc.scalar.activation(xrr, xres[:, ci, njs], AF.Identity)
                        nc.tensor.matmul(s_mu, ones_invC, xrr,
                                         start=(ci == 0), stop=(ci == CT - 1))
                    s_sq = ps_row.tile([1, 512], F32, tag="row")
                    for ci in range(CT):
                        sqt = sq_p.tile([128, 512], F32R, tag="sq")
                        nc.vector.tensor_mul(sqt, xres[:, ci, njs],
                                             xres[:, ci, njs])
                        nc.tensor.matmul(s_sq, ones_invC, sqt,
                                         start=(ci == 0), stop=(ci == CT - 1))
                    t_mu = rows_p.tile([1, 512], F32, tag="rows")
                    nc.vector.tensor_copy(t_mu, s_mu)
                    t_var = rows_p.tile([1, 512], F32R, tag="rows")
                    nc.vector.tensor_mul(t_var, t_mu, t_mu)
                    nc.vector.tensor_sub(t_var, s_sq, t_var)
                    t_rstd = rows_p.tile([1, 512], F32R, tag="rows")
                    nc.scalar.activation(t_rstd, t_var, AF.Sqrt, bias=eps_t)
                    with nc.allow_low_precision(reason="f32r rstd"):
                        nc.vector.reciprocal(t_var, t_rstd)      # t_var = rstd
                    nc.vector.tensor_mul(t_rstd, t_mu, t_var)    # mu * rstd
                    bc_r = ps_row.tile([128, 512], F32, tag="row")
                    nc.tensor.matmul(bc_r, ones65[0:1, :], t_var[0:1, :],
                                     start=True, stop=True)
                    bc_mr = ps_row.tile([128, 512], F32, tag="row")
                    nc.tensor.matmul(bc_mr, ones65[0:1, :], t_rstd[0:1, :],
                                     start=True, stop=True)
                    for ci in range(CT):
                        t = dst[:, ci, njs]
                        nc.vector.tensor_mul(t, xres[:, ci, njs], bc_r)
                        nc.vector.tensor_sub(t, t, bc_mr)
                        nc.scalar.activation(
                            t, t, AF.Identity,
                            scale=sp_tile[:, ci:ci + 1],
                            bias=modsb[:, sh_off + ci:sh_off + ci + 1])

            def transpose_rows(dst, src_dram, row0, nrows):
                """dst[:, ci, rr*128 + f] = src[(row0+rr)*128 + f, ci*128 + p]"""
                for rr_ in range(nrows):
                    natw = wnat_p.tile([128, C], F32R, tag="wnat")
                    dma_rr(natw,
                           r(src_dram[(row0 + rr_) * 128:(row0 + rr_ + 1) * 128, :]))
                    for cp in range(CT // PK):
                        pst = ps_tp.tile([128, 128 * PK], F32R, tag="tp")
                        for k in range(PK):
                            ci = cp * PK + k
                            nc.tensor.matmul(pst[:, k * 128:(k + 1) * 128],
                                             natw[:, ci * 128:(ci + 1) * 128],
                                             ident_r, is_transpose=True,
                                             start=True, stop=True)
                        dsl = dst[:, cp * PK:(cp + 1) * PK,
                                  rr_ * 128:(rr_ + 1) * 128]
                        psr = pst.rearrange("p (a b) -> p a b", a=PK)
                        if (rr_ + cp) % 2 == 0:
                            nc.vector.tensor_copy(dsl, psr)
                        else:
                            nc.scalar.activation(dsl, psr, AF.Identity)

            # ================= attention =================
            if not _stop("x"):
              y = sb.tile([128, CT, N], F32R, tag="lnout")
              layer_norm(y, 0, sp_msa)
            if not _stop("x") and not _stop("ln1"):

              oT = sb.tile([128, CT, N], BF16, tag="oT")

              for g in range(NHG):
                  # ---- q,k (channel-major) ----
                  qkT = sb.tile([128, HG, N], F32R, tag="big")
                  for sl in range(HG):
                      isq = sl < HG // 2
                      fi = (g * (HG // 2) + sl) if isq else (
                          CT + g * (HG // 2) + (sl - HG // 2))
                      wtt = wt_p.tile([128, CT, 128], F32R, tag="wt")
                      transpose_rows(wtt, qkvw_d, fi, 1)
                      for nj in range(NJ):
                          njs = slice(nj * 512, (nj + 1) * 512)
                          pmm = ps_mm.tile([128, 512], F32, tag="mm")
                          for ci in range(CT):
                              nc.tensor.matmul(pmm, wtt[:, ci, :],
                                               y[:, ci, njs],
                                               start=(ci == 0), stop=(ci == CT - 1))
                          nc.scalar.activation(qkT[:, sl, njs], pmm, AF.Identity,
                                               bias=qkvb_t[:, fi:fi + 1])

                  if g == 0:
                      # remaining modulation vectors; overlaps attention compute
                      ada_part(range(2 * CT, 6 * CT))
                      sp_mlp = consts.tile([128, CT], F32, tag="spmlp")
                      nc.vector.tensor_scalar(sp_mlp, modsb[:, 4 * CT:5 * CT],
                                              1.0, None, OP.add)
                      bg1 = consts.tile([128, CT], F32, tag="bg1")
                      nc.vector.tensor_mul(bg1, modsb[:, 2 * CT:3 * CT], projb_t)
                      bg2 = consts.tile([128, CT], F32, tag="bg2")
                      nc.vector.tensor_mul(bg2, modsb[:, 5 * CT:6 * CT], fc2b_t)

                  # ---- v (token-major, ones-augmented) ----
                  vaug = sb.tile([128, NT, HG, 65], BF16, tag="vaug")
                  nc.gpsimd.memset(vaug[:, :, :, 64:65], 1.0)
                  for vg in range((HG * 64 + 255) // 256):
                      wtt = wtv_p.tile([128, CT, 256], F32R, tag="wtv")
                      transpose_rows(wtt, qkvw_d, 2 * CT + g * (HG // 2) + vg * 2, 2)
                      vbrow = rows_p.tile([1, 512], F32R, tag="rows")
                      off = 2 * C + (g * HG + vg * 4) * 64
                      nc.sync.dma_start(
                          out=vbrow[0:1, 0:256],
                          in_=r(qkvb_d[off:off + 256]).rearrange("(a c) -> a c",
                                                                 a=1))
                      vbp = ps_row.tile([128, 512], F32, tag="row")
                      nc.tensor.matmul(vbp[:, 0:256], ones65[0:1, :],
                                       vbrow[0:1, 0:256], start=True, stop=True)
                      vb = bc_p.tile([128, 512], F32, tag="bc")
                      nc.vector.tensor_copy(vb[:, 0:256], vbp[:, 0:256])
                      for ni in range(NT):
                          pv = ps_mm.tile([128, 512], F32, tag="mm")
                          for ci in range(CT):
                              nc.tensor.matmul(
                                  pv[:, 0:256],
                                  y[:, ci, ni * 128:(ni + 1) * 128],
                                  wtt[:, ci, :],
                                  start=(ci == 0), stop=(ci == CT - 1))
                          nc.vector.tensor_add(
                              vaug[:, ni, vg * 4:vg * 4 + 4, 0:64],
                              pv[:, 0:256].rearrange("p (a b) -> p a b", a=4),
                              vb[:, 0:256].rearrange("p (a b) -> p a b", a=4))

                  # ---- attention proper ----
                  for nj in range(NJ):
                      njs = slice(nj * 512, (nj + 1) * 512)
                      mi_hi = min(NT, 4 * (nj + 1))
                      for hp in range(HG // 2):
                          qsl, ksl = hp, HG // 2 + hp
                          po0 = ps_o.tile([65, 512], F32, tag="po")
                          po1 = ps_o.tile([65, 512], F32, tag="po")
                          pos = [po0, po1]
                          for mi in range(mi_hi):
                              delta = 128 * mi - 512 * nj
                              pts = []
                              sps = []
                              for sub in range(2):
                                  base = sub * 64
                                  ps_s = ps_mm.tile([128, 512], F32,
                                                    tag="mm", name="s%d" % sub)
                                  # pair shares PE via disjoint row groups
                                  nc.tensor.matmul(
                                      ps_s,
                                      qkT[base:base + 64, ksl,
                                          mi * 128:(mi + 1) * 128],
                                      qkT[base:base + 64, qsl, njs],
                                      start=True, stop=True)
                                  sps.append(ps_s)
                              for sub in range(2):
                                  pt = pt_p.tile([128, 512], BF16,
                                                 tag="pt", name="pt%d" % sub)
                                  if delta < 0:
                                      nc.scalar.activation(pt, sps[sub],
                                                           AF.Exp,
                                                           scale=D ** -0.5)
                                  else:
                                      if delta > 0:
                                          nc.gpsimd.memset(pt[:, 0:delta], 0.0)
                                      nc.scalar.activation(
                                          pt[:, delta:512],
                                          sps[sub][:, delta:512],
                                          AF.Exp, scale=D ** -0.5)
                                      band = min(128, 512 - delta)
                                      nc.vector.tensor_mul(
                                          pt[:, delta:delta + band],
                                          pt[:, delta:delta + band],
                                          masks[:, delta // 128,
                                                delta:delta + band])
                                  pts.append(pt)
                              for sub in range(2):
                                  hl = 2 * hp + sub
                                  nc.tensor.matmul(pos[sub],
                                                   vaug[:, mi, hl, :],
                                                   pts[sub],
                                                   start=(mi == 0),
                                                   stop=(mi == mi_hi - 1))
                          for sub in range(2):
                              hl = 2 * hp + sub
                              h_glob = g * HG + hl
                              po = pos[sub]
                              srow = rows_p.tile([65, 512], F32R, tag="rows")
                              with nc.allow_low_precision(reason="f32r recip"):
                                  nc.vector.reciprocal(srow[64:65, :],
                                                       po[64:65, :])
                              rbp = ps_row.tile([128, 512], F32, tag="row")
                              nc.tensor.matmul(rbp[:, :], ones65[64:65, :],
                                               srow[64:65, :],
                                               start=True, stop=True)
                              rb = bc_p.tile([128, 512], F32, tag="bc")
                              nc.vector.tensor_copy(rb[0:64, :], rbp[0:64, :])
                              if sub == 0:
                                  nc.vector.tensor_mul(
                                      oT[0:64, h_glob // 2, njs],
                                      po[0:64, :], rb[0:64, :])
                              else:
                                  tsh = pt_p.tile([128, 512], BF16, tag="pt")
                                  nc.vector.tensor_mul(tsh[0:64, :],
                                                       po[0:64, :],
                                                       rb[0:64, :])
                                  nc.sync.dma_start(
                                      out=oT[64:128, h_glob // 2, njs],
                                      in_=tsh[0:64, :])

            if stop_after is None:
              # ================= proj + gated residual =================
              for jc in range(CT):
                  wtt = wt_p.tile([128, CT, 128], BF16, tag="wt")
                  transpose_rows(wtt, projw_d, jc, 1)
                  for nj in range(NJ):
                      njs = slice(nj * 512, (nj + 1) * 512)
                      pmm = ps_mm.tile([128, 512], F32, tag="mm")
                      for ci in range(CT):
                          nc.tensor.matmul(pmm, wtt[:, ci, :], oT[:, ci, njs],
                                           start=(ci == 0), stop=(ci == CT - 1))
                      t = sq_p.tile([128, 512], F32, tag="sq")
                      nc.vector.tensor_scalar(
                          t, pmm, modsb[:, 2 * CT + jc:2 * CT + jc + 1],
                          bg1[:, jc:jc + 1], OP.mult, OP.add)
                      nc.vector.tensor_add(xres[:, jc, njs], xres[:, jc, njs], t)

              # ================= MLP =================
              z2 = sb.tile([128, CT, N], F32R, tag="lnout")
              layer_norm(z2, 3 * CT, sp_mlp)

              if NG > 1:
                  m2acc = sb.tile([128, CT, N], F32, tag="vaug")
              else:
                  m2acc = None

              for gi in range(NG):
                  h_t = sb.tile([128, G, NJ, 512], F32R, tag="big")
                  w2tg = sb.tile([128, G, CT, 128], F32R, tag="oT")
                  for dl in range(G):
                      dd = gi * G + dl
                      w1t = wt_p.tile([128, CT, 128], F32R, tag="wt")
                      transpose_rows(w1t, fc1w_d, dd, 1)
                      for nj in range(NJ):
                          njs = slice(nj * 512, (nj + 1) * 512)
                          ph = ps_mm.tile([128, 512], F32, tag="mm")
                          for ci in range(CT):
                              nc.tensor.matmul(ph, w1t[:, ci, :],
                                               z2[:, ci, njs],
                                               start=(ci == 0), stop=(ci == CT - 1))
                          if not sim_safe:
                              nc.scalar.activation(h_t[:, dl, nj, :], ph,
                                                   AF.Gelu_apprx_tanh,
                                                   bias=fc1b_t[:, dd:dd + 1])
                          else:
                              # gelu_tanh decomposed for CoreSim (no Gelu impl)
                              s2pi = float(np.sqrt(2.0 / np.pi))
                              hs = sq_p.tile([128, 512], F32, tag="sq")
                              nc.scalar.activation(hs, ph, AF.Identity,
                                                   bias=fc1b_t[:, dd:dd + 1])
                              hq = sq_p.tile([128, 512], F32, tag="sq")
                              nc.scalar.activation(hq, ph, AF.Square,
                                                   bias=fc1b_t[:, dd:dd + 1])
                              nc.vector.tensor_scalar(hq, hq, s2pi * 0.044715,
                                                      s2pi, OP.mult, OP.add)
                              nc.vector.tensor_mul(hq, hq, hs)
                              nc.scalar.activation(hq, hq, AF.Tanh)
                              nc.vector.tensor_scalar(hq, hq, 0.5, 0.5,
                                                      OP.mult, OP.add)
                              nc.vector.tensor_mul(h_t[:, dl, nj, :], hq, hs)
                      natc = wnat_p.tile([128, C], F32R, tag="wnat")
                      dma_rr(natc.rearrange("p (a b) -> p a b", a=CT),
                             r(fc2w_d.ap().rearrange("(a p) d -> p a d", p=128)
                               [:, :, dd * 128:(dd + 1) * 128]))
                      for cp in range(CT // PK):
                          pst = ps_tp.tile([128, 128 * PK], F32R, tag="tp")
                          for k in range(PK):
                              jc = cp * PK + k
                              nc.tensor.matmul(
                                  pst[:, k * 128:(k + 1) * 128],
                                  natc[:, jc * 128:(jc + 1) * 128], ident_r,
                                  is_transpose=True, start=True, stop=True)
                          nc.vector.tensor_copy(
                              w2tg[:, dl, cp * PK:(cp + 1) * PK, :],
                              pst.rearrange("p (a b) -> p a b", a=PK))
                  for jc in range(CT):
                      for nj in range(NJ):
                          njs = slice(nj * 512, (nj + 1) * 512)
                          pm = ps_mm.tile([128, 512], F32, tag="mm")
                          for dl in range(G):
                              nc.tensor.matmul(pm, w2tg[:, dl, jc, :],
                                               h_t[:, dl, nj, :],
                                               start=(dl == 0), stop=(dl == G - 1))
                          if NG > 1 and gi == 0:
                              nc.vector.tensor_copy(m2acc[:, jc, njs], pm)
                          elif gi < NG - 1:
                              nc.vector.tensor_add(m2acc[:, jc, njs],
                                                   m2acc[:, jc, njs], pm)
                          else:
                              t = sq_p.tile([128, 512], F32, tag="sq")
                              if NG > 1:
                                  nc.vector.tensor_add(t, m2acc[:, jc, njs], pm)
                              else:
                                  nc.vector.tensor_copy(t, pm)
                              nc.vector.tensor_scalar(
                                  t, t, modsb[:, 5 * CT + jc:5 * CT + jc + 1],
                                  bg2[:, jc:jc + 1], OP.mult, OP.add)
                              nc.vector.tensor_add(xres[:, jc, njs],
                                                   xres[:, jc, njs], t)

            # ================= write out =================
            for ni in range(NT):
                onat = wnat_p.tile([128, C], F32, tag="wnat")
                for cp in range(CT // PK):
                    pst = ps_tp.tile([128, 128 * PK], F32, tag="tp")
                    for k in range(PK):
                        ci = cp * PK + k
                        nc.tensor.matmul(pst[:, k * 128:(k + 1) * 128],
                                         xres[:, ci, ni * 128:(ni + 1) * 128],
                                         ident, is_transpose=True,
                                         start=True, stop=True)
                    nc.vector.tensor_copy(
                        onat[:, cp * 128 * PK:(cp + 1) * 128 * PK], pst)
                nc.sync.dma_start(out=out_d[ni * 128:(ni + 1) * 128, :], in_=onat)

    if not nc.is_finalized():
        nc.finalize()
    return nc


_CACHE = {}


def get_program(key="full", **kw):
    if key not in _CACHE:
        _CACHE[key] = build_program(**kw)
    return _CACHE[key]


def make_in_maps(inputs):
    x = np.ascontiguousarray(np.asarray(inputs["x"], np.float32))
    c = np.ascontiguousarray(np.asarray(inputs["c"], np.float32))
    B = x.shape[0]
    shared = {k: np.ascontiguousarray(np.asarray(inputs[k], np.float32))
              for k in ("qkv_w", "qkv_b", "proj_w", "proj_b", "fc1_w",
                        "fc1_b", "fc2_w", "fc2_b", "ada_w", "ada_b")}
    return [dict(shared, x=x[b], cvec=c[b, 0]) for b in range(B)]


def kernel(**inputs):
    from concourse.bass_utils import run_bass_kernel_spmd

    x = np.asarray(inputs["x"])
    B, N, C = x.shape
    nc = get_program("full", N=N, C=C, H=16, DFF=4 * C, head_group=8,
                     dff_group=4, n_cores=B)
    in_maps = make_in_maps(inputs)
    res = run_bass_kernel_spmd(nc, in_maps, core_ids=list(range(B)))
    out = np.stack([res.results[b]["out"] for b in range(B)], axis=0)
    return out.astype(np.float32)



# revision 2
# speedup vs baseline: 1.0004x; 1.0004x over previous
"""DiT block kernel for Trainium2 (Bass/Tile), 8-core data-parallel, v2.

Changes vs v1 baseline:
  - bf16 weights + activations in all GEMM paths (fp32 residual + PSUM
    accumulation kept); enables FWL fast weight loads.
  - LN stats read xres via f32r bitcast (no ACT cast pass).
  - Attention inner loop software-pipelined: score matmul of mi+1 issues
    before the accumulate matmul of mi, hiding the exp latency; score
    tiles round-robin across two PSUM pools for 4 banks in flight.
  - dff_group=8: longer fc2 accumulation chains, fewer partial passes.
  - Write-out fused into the last MLP group (nj-outer loop).
  - DMAs issued from HWDGE engines only (sync/scalar).
"""

import sys

sys.path.insert(0, "/opt/trn_rl_repo")

import numpy as np

import concourse.bass as bass
import concourse.bacc as bacc
import concourse.mybir as mybir
from concourse import library_config
from concourse.tile import TileContext

F32 = mybir.dt.float32
F32R = mybir.dt.float32r
BF16 = mybir.dt.bfloat16
AF = mybir.ActivationFunctionType
OP = mybir.AluOpType


def r(ap):
    return ap.bitcast(F32R)


def build_program(N=1024, C=1024, H=16, DFF=4096, head_group=8, dff_group=8,
                  n_cores=8, sim_safe=False, stop_after=None, loop_iters=None):
    D = 64
    NT, CT, DT = N // 128, C // 128, DFF // 128
    NJ = N // 512
    HG = head_group
    NHG = H // HG
    G = dff_group
    NG = DT // G
    PK = min(4, CT)            # transpose pack (blocks per psum tile)
    assert H % HG == 0 and DT % G == 0 and N % 512 == 0 and CT % PK == 0
    assert HG % 2 == 0 and D == 64

    nc = bacc.Bacc("TRN2", target_bir_lowering=False, debug=False,
                   num_devices=n_cores, num_swdge_queues=4)

    x_d = nc.dram_tensor("x", [N, C], F32, kind="ExternalInput")
    c_d = nc.dram_tensor("cvec", [C], F32, kind="ExternalInput")
    qkvw_d = nc.dram_tensor("qkv_w", [3 * C, C], F32, kind="ExternalInput")
    qkvb_d = nc.dram_tensor("qkv_b", [3 * C], F32, kind="ExternalInput")
    projw_d = nc.dram_tensor("proj_w", [C, C], F32, kind="ExternalInput")
    projb_d = nc.dram_tensor("proj_b", [C], F32, kind="ExternalInput")
    fc1w_d = nc.dram_tensor("fc1_w", [DFF, C], F32, kind="ExternalInput")
    fc1b_d = nc.dram_tensor("fc1_b", [DFF], F32, kind="ExternalInput")
    fc2w_d = nc.dram_tensor("fc2_w", [C, DFF], F32, kind="ExternalInput")
    fc2b_d = nc.dram_tensor("fc2_b", [C], F32, kind="ExternalInput")
    adaw_d = nc.dram_tensor("ada_w", [6 * C, C], F32, kind="ExternalInput")
    adab_d = nc.dram_tensor("ada_b", [6 * C], F32, kind="ExternalInput")
    out_d = nc.dram_tensor("out", [N, C], F32, kind="ExternalOutput")

    from contextlib import ExitStack
    with TileContext(nc) as tc, ExitStack() as ctx:
        consts = ctx.enter_context(tc.tile_pool(name="consts", bufs=1))
        sb = ctx.enter_context(tc.tile_pool(name="sb", bufs=1))
        wnat_p = ctx.enter_context(tc.tile_pool(name="wnat", bufs=3))
        ada_p = ctx.enter_context(tc.tile_pool(name="adap", bufs=4))
        prod_p = ctx.enter_context(tc.tile_pool(name="prodp", bufs=3))
        wt_p = ctx.enter_context(tc.tile_pool(name="wt", bufs=2))
        wtv_p = ctx.enter_context(tc.tile_pool(name="wtv", bufs=1))
        pt_p = ctx.enter_context(tc.tile_pool(name="pt", bufs=4))
        rows_p = ctx.enter_context(tc.tile_pool(name="rows", bufs=3))
        bc_p = ctx.enter_context(tc.tile_pool(name="bc", bufs=2))
        sq_p = ctx.enter_context(tc.tile_pool(name="sqp", bufs=3))

        ps_tp = ctx.enter_context(
            tc.tile_pool(name="ps_tp", bufs=2, space="PSUM"))
        ps_mm = ctx.enter_context(
            tc.tile_pool(name="ps_mm", bufs=2, space="PSUM"))
        ps_row = ctx.enter_context(
            tc.tile_pool(name="ps_row", bufs=1, space="PSUM"))
        ps_o = ctx.enter_context(
            tc.tile_pool(name="ps_o", bufs=3, space="PSUM"))

        from contextlib import nullcontext
        loop_cm = tc.For_i(0, loop_iters, 1) if loop_iters else nullcontext()
        with loop_cm:
            _dmac = [0]

            def dma_rr(out, in_):
                i = _dmac[0]; _dmac[0] += 1
                eng = (nc.sync, nc.scalar)[i % 2]
                eng.dma_start(out=out, in_=in_)
            ident = consts.tile([128, 128], F32, tag="ident")
            nc.gpsimd.memset(ident, 0.0)
            nc.gpsimd.affine_select(
                out=ident, in_=ident, compare_op=OP.not_equal, fill=1.0,
                base=0, pattern=[[-1, 128]], channel_multiplier=1)
            # ================= standing tensors =================
            # F32R so LN stat matmuls can consume it directly (BIR requires
            # f32r matmul inputs to be produced as f32r, not bitcast).
            xres = sb.tile([128, CT, N], F32R, tag="xres")

            for ni in range(NT):
                natx = wnat_p.tile([128, C], F32, tag="wnat")
                dma_rr(natx, x_d[ni * 128:(ni + 1) * 128, :])
                for cp in range(CT // PK):
                    pst = ps_tp.tile([128, 128 * PK], F32, tag="tp")
                    for k in range(PK):
                        ci = cp * PK + k
                        nc.tensor.matmul(pst[:, k * 128:(k + 1) * 128],
                                         natx[:, ci * 128:(ci + 1) * 128], ident,
                                         is_transpose=True, start=True, stop=True)
                    dsl = xres[:, cp * PK:(cp + 1) * PK, ni * 128:(ni + 1) * 128]
                    psr = pst.rearrange("p (a b) -> p a b", a=PK)
                    if (ni + cp) % 2 == 0:
                        nc.vector.tensor_copy(dsl, psr)
                    else:
                        nc.scalar.activation(dsl, psr, AF.Identity)

            def _stop(tag_):
                return stop_after is not None and stop_after == tag_

            # ================= constants =================
            ident_r = consts.tile([128, 128], F32R, tag="identr")
            nc.vector.tensor_copy(ident_r, ident)
            ident_b = consts.tile([128, 128], BF16, tag="identb")
            nc.vector.tensor_copy(ident_b, ident)
            ones65f = wnat_p.tile([65, 128], F32, tag="bnat")
            nc.vector.memset(ones65f, 1.0)
            ones65 = consts.tile([65, 128], F32R, tag="ones65")
            nc.vector.tensor_copy(ones65, ones65f)

            masks = consts.tile([128, 4, 512], BF16, tag="masks")
            nc.gpsimd.memset(masks, 1.0)
            for i in range(4):
                # keep where n >= m  <=>  s - r - delta >= 0 (delta = 128i)
                nc.gpsimd.affine_select(
                    out=masks[:, i, :], in_=masks[:, i, :], compare_op=OP.is_ge,
                    fill=0.0, base=-(128 * i), pattern=[[1, 512]],
                    channel_multiplier=-1)

            ones_invCf = consts.tile([128, 1], F32, tag="onescf")
            nc.gpsimd.memset(ones_invCf, 1.0 / C)
            ones_invC = consts.tile([128, 1], F32R, tag="onesc")
            nc.vector.tensor_copy(ones_invC, ones_invCf)
            eps_t = consts.tile([1, 1], F32, tag="eps")
            nc.vector.memset(eps_t, 1e-6)

            def bias_T(src, nch, tag):
                t = consts.tile([128, nch], F32, tag=tag)
                natb = wnat_p.tile([nch, 128], F32, tag="bnat")
                nc.sync.dma_start(out=natb,
                                  in_=src.rearrange("(a b) -> a b", b=128))
                pst = ps_tp.tile([128, 128 * PK], F32, tag="tp")
                nc.tensor.matmul(pst[:, 0:nch], natb, ident[0:nch, 0:nch],
                                 is_transpose=True, start=True, stop=True)
                nc.vector.tensor_copy(t, pst[:, 0:nch])
                return t

            qkvb_t = bias_T(qkvb_d.ap(), 3 * CT, "qkvbt")
            projb_t = bias_T(projb_d.ap(), CT, "projbt")
            fc1b_t = bias_T(fc1b_d.ap(), DT, "fc1bt")
            fc2b_t = bias_T(fc2b_d.ap(), CT, "fc2bt")
            adab_t = bias_T(adab_d.ap(), 6 * CT, "adabt")

            # ---- adaLN: silu(c) broadcast ----
            crow = wnat_p.tile([1, C], F32, tag="wnat")
            nc.sync.dma_start(out=crow, in_=c_d.ap().rearrange("(a c) -> a c", a=1))
            silu_row = wnat_p.tile([1, C], F32R, tag="wnat")
            nc.scalar.activation(silu_row, crow, AF.Sigmoid)
            nc.vector.tensor_mul(silu_row, silu_row, crow)
            silu_b = sb.tile([128, C], F32, tag="silub")
            for w0 in range(0, C, 512):
                w = min(512, C - w0)
                pb = ps_row.tile([128, 512], F32, tag="row")
                nc.tensor.matmul(pb[:, 0:w], ones65[0:1, :],
                                 silu_row[0:1, w0:w0 + w], start=True, stop=True)
                nc.vector.tensor_copy(silu_b[:, w0:w0 + w], pb[:, 0:w])

            modsb = consts.tile([128, 6 * CT], F32, tag="modsb")
            adadump = consts.tile([128, C], F32, tag="adadump")

            silu_bf = sb.tile([128, C], BF16, tag="silubf")
            nc.vector.tensor_copy(silu_bf, silu_b)

            def ada_block(jt):
                anat = ada_p.tile([128, C], F32, tag="ada")
                # SWDGE queue: keeps bulk ada traffic off the
                # latency-critical HWDGE weight rings
                nc.gpsimd.dma_start(out=anat,
                                    in_=adaw_d[jt * 128:(jt + 1) * 128, :])
                nc.vector.tensor_mul(anat, anat, silu_b)
                nc.scalar.activation(adadump, anat, AF.Identity,
                                     accum_out=modsb[:, jt:jt + 1])

            def ada_bias(lo, hi):
                nc.vector.tensor_add(modsb[:, lo:hi], modsb[:, lo:hi],
                                     adab_t[:, lo:hi])

            # shift/scale_msa now (LN1 path); the remaining 4 chunks are
            # interleaved into attention below so their ACT accumulates don't
            # head-of-line block attention's exps on the ACT queue.
            for jt in range(2 * CT):
                ada_block(jt)
            ada_bias(0, 2 * CT)
            sp_msa = consts.tile([128, CT], F32, tag="spmsa")
            nc.vector.tensor_scalar(sp_msa, modsb[:, CT:2 * CT], 1.0, None, OP.add)
            ada_todo = list(range(2 * CT, 6 * CT))

            def ada_step(n):
                for _ in range(min(n, len(ada_todo))):
                    ada_block(ada_todo.pop(0))

            # ================= helpers =================
            def layer_norm(dst, sh_off, sp_tile):
                for nj in range(NJ):
                    njs = slice(nj * 512, (nj + 1) * 512)
                    s_mu = ps_row.tile([1, 512], F32, tag="row")
                    for ci in range(CT):
                        nc.tensor.matmul(s_mu, ones_invC, xres[:, ci, njs],
                                         start=(ci == 0), stop=(ci == CT - 1))
                    s_sq = ps_row.tile([1, 512], F32, tag="row")
                    for ci in range(CT):
                        sqt = sq_p.tile([128, 512], F32R, tag="sq")
                        nc.scalar.activation(sqt, xres[:, ci, njs], AF.Square)
                        nc.tensor.matmul(s_sq, ones_invC, sqt,
                                         start=(ci == 0), stop=(ci == CT - 1))
                    t_mu = rows_p.tile([1, 512], F32, tag="rows")
                    nc.vector.tensor_copy(t_mu, s_mu)
                    t_var = rows_p.tile([1, 512], F32R, tag="rows")
                    nc.vector.tensor_mul(t_var, t_mu, t_mu)
                    nc.vector.tensor_sub(t_var, s_sq, t_var)
                    t_rstd = rows_p.tile([1, 512], F32R, tag="rows")
                    nc.scalar.activation(t_rstd, t_var, AF.Sqrt, bias=eps_t)
                    with nc.allow_low_precision(reason="f32r rstd"):
                        nc.vector.reciprocal(t_var, t_rstd)      # t_var = rstd
                    nc.vector.tensor_mul(t_rstd, t_mu, t_var)    # mu * rstd
                    bc_rp = ps_row.tile([128, 512], F32, tag="row")
                    nc.tensor.matmul(bc_rp, ones65[0:1, :], t_var[0:1, :],
                                     start=True, stop=True)
                    bc_r = bc_p.tile([128, 512], F32, tag="bc")
                    nc.vector.tensor_copy(bc_r, bc_rp)
                    bc_mrp = ps_row.tile([128, 512], F32, tag="row")
                    nc.tensor.matmul(bc_mrp, ones65[0:1, :], t_rstd[0:1, :],
                                     start=True, stop=True)
                    bc_mr = bc_p.tile([128, 512], F32, tag="bc")
                    nc.vector.tensor_copy(bc_mr, bc_mrp)
                    for ci in range(CT):
                        t = sq_p.tile([128, 512], F32, tag="sq")
                        nc.vector.tensor_mul(t, xres[:, ci, njs], bc_r)
                        nc.vector.tensor_sub(t, t, bc_mr)
                        nc.scalar.activation(
                            dst[:, ci, njs], t, AF.Identity,
                            scale=sp_tile[:, ci:ci + 1],
                            bias=modsb[:, sh_off + ci:sh_off + ci + 1])

            def transpose_rows(dst, src_dram, row0, nrows, hwdge=False):
                """dst[:, ci, rr*128 + f] = src[(row0+rr)*128 + f, ci*128 + p]

                Default path: SWDGE casts f32->bf16 in flight, PE transpose
                at 1 cyc/row, evac at 2x.  hwdge=True keeps the load on the
                HWDGE rings (f32) for latency-critical weights so they do
                not queue behind bulk SWDGE emissions on the Pool engine."""
                for rr_ in range(nrows):
                    natw = wnat_p.tile([128, C], F32R, tag="wnat")
                    dma_rr(natw, r(src_dram[(row0 + rr_) * 128:
                                           (row0 + rr_ + 1) * 128, :]))
                    pdt, idt = F32R, ident_r
                    for cp in range(CT // PK):
                        pst = ps_tp.tile([128, 128 * PK], pdt, tag="tp")
                        for k in range(PK):
                            ci = cp * PK + k
                            nc.tensor.matmul(pst[:, k * 128:(k + 1) * 128],
                                             natw[:, ci * 128:(ci + 1) * 128],
                                             idt, is_transpose=True,
                                             start=True, stop=True)
                        dsl = dst[:, cp * PK:(cp + 1) * PK,
                                  rr_ * 128:(rr_ + 1) * 128]
                        psr = pst.rearrange("p (a b) -> p a b", a=PK)
                        if (rr_ + cp) % 2 == 0:
                            nc.vector.tensor_copy(dsl, psr)
                        else:
                            nc.scalar.activation(dsl, psr, AF.Identity)

            # ================= attention =================
            if not _stop("x"):
              y = sb.tile([128, CT, N], BF16, tag="lnout")
              layer_norm(y, 0, sp_msa)
            if not _stop("x") and not _stop("ln1"):

              oT = sb.tile([128, CT, N], BF16, tag="oT")

              for g in range(NHG):
                  # ---- q,k (channel-major) ----
                  qkT = sb.tile([128, HG, N], BF16, tag="big")
                  for sl in range(HG):
                      isq = sl < HG // 2
                      fi = (g * (HG // 2) + sl) if isq else (
                          CT + g * (HG // 2) + (sl - HG // 2))
                      wtt = wt_p.tile([128, CT, 128], BF16, tag="wt")
                      transpose_rows(wtt, qkvw_d, fi, 1, hwdge=True)
                      for nj in range(NJ):
                          njs = slice(nj * 512, (nj + 1) * 512)
                          pmm = ps_mm.tile([128, 512], F32, tag="mm")
                          for ci in range(CT):
                              nc.tensor.matmul(pmm, wtt[:, ci, :],
                                               y[:, ci, njs],
                                               start=(ci == 0), stop=(ci == CT - 1))
                          nc.scalar.activation(qkT[:, sl, njs], pmm, AF.Identity,
                                               bias=qkvb_t[:, fi:fi + 1])

                  # ---- v (token-major, ones-augmented) ----
                  vaug = sb.tile([128, NT, HG, 65], BF16, tag="vaug")
                  nc.gpsimd.memset(vaug[:, :, :, 64:65], 1.0)
                  for vg in range((HG * 64 + 255) // 256):
                      wtt = wtv_p.tile([128, CT, 256], BF16, tag="wtv")
                      transpose_rows(wtt, qkvw_d, 2 * CT + g * (HG // 2) + vg * 2,
                                     2, hwdge=True)
                      vbrow = rows_p.tile([1, 512], F32R, tag="rows")
                      off = 2 * C + (g * HG + vg * 4) * 64
                      nc.sync.dma_start(
                          out=vbrow[0:1, 0:256],
                          in_=r(qkvb_d[off:off + 256]).rearrange("(a c) -> a c",
                                                                 a=1))
                      vbp = ps_row.tile([128, 512], F32, tag="row")
                      nc.tensor.matmul(vbp[:, 0:256], ones65[0:1, :],
                                       vbrow[0:1, 0:256], start=True, stop=True)
                      vb = bc_p.tile([128, 512], F32, tag="bc")
                      nc.vector.tensor_copy(vb[:, 0:256], vbp[:, 0:256])
                      for ni in range(NT):
                          pv = ps_mm.tile([128, 512], F32, tag="mm")
                          for ci in range(CT):
                              nc.tensor.matmul(
                                  pv[:, 0:256],
                                  y[:, ci, ni * 128:(ni + 1) * 128],
                                  wtt[:, ci, :],
                                  start=(ci == 0), stop=(ci == CT - 1))
                          nc.vector.tensor_add(
                              vaug[:, ni, vg * 4:vg * 4 + 4, 0:64],
                              pv[:, 0:256].rearrange("p (a b) -> p a b", a=4),
                              vb[:, 0:256].rearrange("p (a b) -> p a b", a=4))

                  # ---- attention proper (software-pipelined) ----
                  for nj in range(NJ):
                      njs = slice(nj * 512, (nj + 1) * 512)
                      mi_hi = min(NT, 4 * (nj + 1))
                      for hp in range(HG // 2):
                          ada_step(1)
                          qsl, ksl = hp, HG // 2 + hp
                          po0 = ps_o.tile([65, 512], F32, tag="po")
                          po1 = ps_o.tile([65, 512], F32, tag="po")
                          pos = [po0, po1]
                          pts_q = {}
                          for mi in range(mi_hi + 1):
                              if mi == mi_hi // 2:
                                  ada_step(1)
                              if mi < mi_hi:
                                  delta = 128 * mi - 512 * nj
                                  sps = []
                                  spool = ps_mm if mi % 2 == 0 else ps_tp
                                  for sub in range(2):
                                      base = sub * 64
                                      ps_s = spool.tile(
                                          [128, 512], F32,
                                          tag="mm" if spool is ps_mm else "tp",
                                          name="s%d" % sub)
                                      # pair shares PE via disjoint row groups
                                      nc.tensor.matmul(
                                          ps_s,
                                          qkT[base:base + 64, ksl,
                                              mi * 128:(mi + 1) * 128],
                                          qkT[base:base + 64, qsl, njs],
                                          start=True, stop=True)
                                      sps.append(ps_s)
                                  pts = []
                                  for sub in range(2):
                                      pt = pt_p.tile([128, 512], BF16,
                                                     tag="pt", name="pt%d" % sub)
                                      nc.scalar.activation(pt, sps[sub],
                                                           AF.Exp,
                                                           scale=D ** -0.5)
                                      if delta >= 0:
                                          # full-width mask mul (keeps the
                                          # zeroing off the Pool engine)
                                          nc.vector.tensor_mul(
                                              pt, pt, masks[:, delta // 128, :])
                                      pts.append(pt)
                                  pts_q[mi] = pts
                              if mi > 0:
                                  pprev = pts_q.pop(mi - 1)
                                  for sub in range(2):
                                      hl = 2 * hp + sub
                                      nc.tensor.matmul(pos[sub],
                                                       vaug[:, mi - 1, hl, :],
                                                       pprev[sub],
                                                       start=(mi - 1 == 0),
                                                       stop=(mi - 1 == mi_hi - 1))
                          for sub in range(2):
                              hl = 2 * hp + sub
                              h_glob = g * HG + hl
                              po = pos[sub]
                              srow = rows_p.tile([65, 512], F32R, tag="rows")
                              with nc.allow_low_precision(reason="f32r recip"):
                                  nc.vector.reciprocal(srow[64:65, :],
                                                       po[64:65, :])
                              rbp = ps_row.tile([128, 512], F32, tag="row")
                              nc.tensor.matmul(rbp[:, :], ones65[64:65, :],
                                               srow[64:65, :],
                                               start=True, stop=True)
                              rb = bc_p.tile([128, 512], F32, tag="bc")
                              nc.vector.tensor_copy(rb[0:64, :], rbp[0:64, :])
                              if sub == 0:
                                  nc.vector.tensor_mul(
                                      oT[0:64, h_glob // 2, njs],
                                      po[0:64, :], rb[0:64, :])
                              else:
                                  tsh = pt_p.tile([128, 512], BF16, tag="pt")
                                  nc.vector.tensor_mul(tsh[0:64, :],
                                                       po[0:64, :],
                                                       rb[0:64, :])
                                  nc.sync.dma_start(
                                      out=oT[64:128, h_glob // 2, njs],
                                      in_=tsh[0:64, :])
              # drain any leftover ada blocks, then the gate/shift constants
              ada_step(len(ada_todo))
              ada_bias(2 * CT, 6 * CT)
              sp_mlp = consts.tile([128, CT], F32, tag="spmlp")
              nc.vector.tensor_scalar(sp_mlp, modsb[:, 4 * CT:5 * CT],
                                      1.0, None, OP.add)
              bg1 = consts.tile([128, CT], F32, tag="bg1")
              nc.vector.tensor_mul(bg1, modsb[:, 2 * CT:3 * CT], projb_t)
              bg2 = consts.tile([128, CT], F32, tag="bg2")
              nc.vector.tensor_mul(bg2, modsb[:, 5 * CT:6 * CT], fc2b_t)

            if stop_after is None:
              # ================= proj + gated residual =================
              for jc in range(CT):
                  wtt = wt_p.tile([128, CT, 128], BF16, tag="wt")
                  transpose_rows(wtt, projw_d, jc, 1)
                  for nj in range(NJ):
                      njs = slice(nj * 512, (nj + 1) * 512)
                      pmm = ps_mm.tile([128, 512], F32, tag="mm")
                      for ci in range(CT):
                          nc.tensor.matmul(pmm, wtt[:, ci, :], oT[:, ci, njs],
                                           start=(ci == 0), stop=(ci == CT - 1))
                      t = sq_p.tile([128, 512], F32, tag="sq")
                      nc.vector.tensor_scalar(
                          t, pmm, modsb[:, 2 * CT + jc:2 * CT + jc + 1],
                          bg1[:, jc:jc + 1], OP.mult, OP.add)
                      nc.vector.tensor_add(xres[:, jc, njs], xres[:, jc, njs], t)

              # ================= MLP =================
              z2 = sb.tile([128, CT, N], BF16, tag="lnout")
              layer_norm(z2, 3 * CT, sp_mlp)

              if NG > 1:
                  m2acc = sb.tile([128, CT, N], F32, tag="vaug")
              else:
                  m2acc = None

              for gi in range(NG):
                  h_t = sb.tile([128, G, NJ, 512], BF16, tag="big")
                  w2tg = sb.tile([128, G, CT, 128], BF16, tag="oT")
                  for dl in range(G):
                      dd = gi * G + dl
                      w1t = wt_p.tile([128, CT, 128], BF16, tag="wt")
                      transpose_rows(w1t, fc1w_d, dd, 1)
                      for nj in range(NJ):
                          njs = slice(nj * 512, (nj + 1) * 512)
                          ph = ps_mm.tile([128, 512], F32, tag="mm")
                          for ci in range(CT):
                              nc.tensor.matmul(ph, w1t[:, ci, :],
                                               z2[:, ci, njs],
                                               start=(ci == 0), stop=(ci == CT - 1))
                          if not sim_safe:
                              nc.scalar.activation(h_t[:, dl, nj, :], ph,
                                                   AF.Gelu_apprx_tanh,
                                                   bias=fc1b_t[:, dd:dd + 1])
                          else:
                              # gelu_tanh decomposed for CoreSim (no Gelu impl)
                              s2pi = float(np.sqrt(2.0 / np.pi))
                              hs = sq_p.tile([128, 512], F32, tag="sq")
                              nc.scalar.activation(hs, ph, AF.Identity,
                                                   bias=fc1b_t[:, dd:dd + 1])
                              hq = sq_p.tile([128, 512], F32, tag="sq")
                              nc.scalar.activation(hq, ph, AF.Square,
                                                   bias=fc1b_t[:, dd:dd + 1])
                              nc.vector.tensor_scalar(hq, hq, s2pi * 0.044715,
                                                      s2pi, OP.mult, OP.add)
                              nc.vector.tensor_mul(hq, hq, hs)
                              nc.scalar.activation(hq, hq, AF.Tanh)
                              nc.vector.tensor_scalar(hq, hq, 0.5, 0.5,
                                                      OP.mult, OP.add)
                              nc.vector.tensor_mul(h_t[:, dl, nj, :], hq, hs)
                      natc = wnat_p.tile([128, C], F32R, tag="wnat")
                      dma_rr(natc.rearrange("p (a b) -> p a b", a=CT),
                             r(fc2w_d.ap().rearrange("(a p) d -> p a d", p=128)
                               [:, :, dd * 128:(dd + 1) * 128]))
                      for cp in range(CT // PK):
                          pst = ps_tp.tile([128, 128 * PK], F32R, tag="tp")
                          for k in range(PK):
                              jc = cp * PK + k
                              nc.tensor.matmul(
                                  pst[:, k * 128:(k + 1) * 128],
                                  natc[:, jc * 128:(jc + 1) * 128], ident_r,
                                  is_transpose=True, start=True, stop=True)
                          dsl = w2tg[:, dl, cp * PK:(cp + 1) * PK, :]
                          psr = pst.rearrange("p (a b) -> p a b", a=PK)
                          if (dl + cp) % 2 == 0:
                              nc.vector.tensor_copy(dsl, psr)
                          else:
                              nc.scalar.activation(dsl, psr, AF.Identity)
                  last = gi == NG - 1
                  for nj in range(NJ):
                      njs = slice(nj * 512, (nj + 1) * 512)
                      for jc in range(CT):
                          pm = ps_mm.tile([128, 512], F32, tag="mm")
                          for dl in range(G):
                              nc.tensor.matmul(pm, w2tg[:, dl, jc, :],
                                               h_t[:, dl, nj, :],
                                               start=(dl == 0), stop=(dl == G - 1))
                          if NG > 1 and gi == 0:
                              nc.vector.tensor_copy(m2acc[:, jc, njs], pm)
                          elif gi < NG - 1:
                              nc.vector.tensor_add(m2acc[:, jc, njs],
                                                   m2acc[:, jc, njs], pm)
                          else:
                              t = sq_p.tile([128, 512], F32, tag="sq")
                              if NG > 1:
                                  nc.vector.tensor_add(t, m2acc[:, jc, njs], pm)
                              else:
                                  nc.vector.tensor_copy(t, pm)
                              nc.vector.tensor_scalar(
                                  t, t, modsb[:, 5 * CT + jc:5 * CT + jc + 1],
                                  bg2[:, jc:jc + 1], OP.mult, OP.add)
                              nc.vector.tensor_add(xres[:, jc, njs],
                                                   xres[:, jc, njs], t)
                      if last:
                          # fused write-out for this nj's 4 token blocks
                          for ni in range(4 * nj, 4 * (nj + 1)):
                              onat = wnat_p.tile([128, C], F32, tag="wnat")
                              for cp in range(CT // PK):
                                  pst = ps_tp.tile([128, 128 * PK], F32R,
                                                   tag="tp")
                                  for k in range(PK):
                                      ci = cp * PK + k
                                      nc.tensor.matmul(
                                          pst[:, k * 128:(k + 1) * 128],
                                          xres[:, ci, ni * 128:(ni + 1) * 128],
                                          ident_r, is_transpose=True,
                                          start=True, stop=True)
                                  if (ni + cp) % 2 == 0:
                                      nc.vector.tensor_copy(
                                          onat[:, cp * 128 * PK:
                                               (cp + 1) * 128 * PK], pst)
                                  else:
                                      nc.scalar.activation(
                                          onat[:, cp * 128 * PK:
                                               (cp + 1) * 128 * PK], pst,
                                          AF.Identity)
                              nc.scalar.dma_start(
                                  out=out_d[ni * 128:(ni + 1) * 128, :],
                                  in_=onat)

            if stop_after is not None:
                # write out whatever xres holds so timing builds stay valid
                for ni in range(NT):
                    onat = wnat_p.tile([128, C], F32, tag="wnat")
                    for cp in range(CT // PK):
                        pst = ps_tp.tile([128, 128 * PK], F32R, tag="tp")
                        for k in range(PK):
                            ci = cp * PK + k
                            nc.tensor.matmul(pst[:, k * 128:(k + 1) * 128],
                                             xres[:, ci, ni * 128:(ni + 1) * 128],
                                             ident_r, is_transpose=True,
                                             start=True, stop=True)
                        nc.vector.tensor_copy(
                            onat[:, cp * 128 * PK:(cp + 1) * 128 * PK], pst)
                    nc.sync.dma_start(out=out_d[ni * 128:(ni + 1) * 128, :],
                                      in_=onat)

    if not nc.is_finalized():
        nc.finalize()
    return nc


_CACHE = {}


def get_program(key="full", **kw):
    if key not in _CACHE:
        _CACHE[key] = build_program(**kw)
    return _CACHE[key]


def make_in_maps(inputs):
    x = np.ascontiguousarray(np.asarray(inputs["x"], np.float32))
    c = np.ascontiguousarray(np.asarray(inputs["c"], np.float32))
    B = x.shape[0]
    shared = {k: np.ascontiguousarray(np.asarray(inputs[k], np.float32))
              for k in ("qkv_w", "qkv_b", "proj_w", "proj_b", "fc1_w",
                        "fc1_b", "fc2_w", "fc2_b", "ada_w", "ada_b")}
    return [dict(shared, x=x[b], cvec=c[b, 0]) for b in range(B)]


def kernel(**inputs):
    from concourse.bass_utils import run_bass_kernel_spmd

    x = np.asarray(inputs["x"])
    B, N, C = x.shape
    nc = get_program("full", N=N, C=C, H=16, DFF=4 * C, n_cores=B)
    in_maps = make_in_maps(inputs)
    res = run_bass_kernel_spmd(nc, in_maps, core_ids=list(range(B)))
    out = np.stack([res.results[b]["out"] for b in range(B)], axis=0)
    return out.astype(np.float32)


# revision 3
# speedup vs baseline: 1.0263x; 1.0259x over previous
"""DiT block kernel for Trainium2 (Bass/Tile), 8-core data-parallel, v2.

Changes vs v1 baseline:
  - bf16 weights + activations in all GEMM paths (fp32 residual + PSUM
    accumulation kept); enables FWL fast weight loads.
  - LN stats read xres via f32r bitcast (no ACT cast pass).
  - Attention inner loop software-pipelined: score matmul of mi+1 issues
    before the accumulate matmul of mi, hiding the exp latency; score
    tiles round-robin across two PSUM pools for 4 banks in flight.
  - dff_group=8: longer fc2 accumulation chains, fewer partial passes.
  - Write-out fused into the last MLP group (nj-outer loop).
  - DMAs issued from HWDGE engines only (sync/scalar).
"""

import sys

sys.path.insert(0, "/opt/trn_rl_repo")

import numpy as np

import concourse.bass as bass
import concourse.bacc as bacc
import concourse.mybir as mybir
from concourse import library_config
from concourse.tile import TileContext

F32 = mybir.dt.float32
F32R = mybir.dt.float32r
BF16 = mybir.dt.bfloat16
AF = mybir.ActivationFunctionType
OP = mybir.AluOpType


def r(ap):
    return ap.bitcast(F32R)


def build_program(N=1024, C=1024, H=16, DFF=4096, head_group=8, dff_group=8,
                  n_cores=8, sim_safe=False, stop_after=None, loop_iters=None):
    D = 64
    NT, CT, DT = N // 128, C // 128, DFF // 128
    NJ = N // 512
    HG = head_group
    NHG = H // HG
    G = dff_group
    NG = DT // G
    PK = min(4, CT)            # transpose pack (blocks per psum tile)
    assert H % HG == 0 and DT % G == 0 and N % 512 == 0 and CT % PK == 0
    assert HG % 2 == 0 and D == 64

    nc = bacc.Bacc("TRN2", target_bir_lowering=False, debug=False,
                   num_devices=n_cores, num_swdge_queues=4)

    x_d = nc.dram_tensor("x", [N, C], F32, kind="ExternalInput")
    c_d = nc.dram_tensor("cvec", [C], F32, kind="ExternalInput")
    qkvw_d = nc.dram_tensor("qkv_w", [3 * C, C], F32, kind="ExternalInput")
    qkvb_d = nc.dram_tensor("qkv_b", [3 * C], F32, kind="ExternalInput")
    projw_d = nc.dram_tensor("proj_w", [C, C], F32, kind="ExternalInput")
    projb_d = nc.dram_tensor("proj_b", [C], F32, kind="ExternalInput")
    fc1w_d = nc.dram_tensor("fc1_w", [DFF, C], F32, kind="ExternalInput")
    fc1b_d = nc.dram_tensor("fc1_b", [DFF], F32, kind="ExternalInput")
    fc2w_d = nc.dram_tensor("fc2_w", [C, DFF], F32, kind="ExternalInput")
    fc2b_d = nc.dram_tensor("fc2_b", [C], F32, kind="ExternalInput")
    adaw_d = nc.dram_tensor("ada_w", [6 * C, C], F32, kind="ExternalInput")
    adab_d = nc.dram_tensor("ada_b", [6 * C], F32, kind="ExternalInput")
    out_d = nc.dram_tensor("out", [N, C], F32, kind="ExternalOutput")

    from contextlib import ExitStack
    with TileContext(nc) as tc, ExitStack() as ctx:
        consts = ctx.enter_context(tc.tile_pool(name="consts", bufs=1))
        sb = ctx.enter_context(tc.tile_pool(name="sb", bufs=1))
        wnat_p = ctx.enter_context(tc.tile_pool(name="wnat", bufs=3))
        ada_p = ctx.enter_context(tc.tile_pool(name="adap", bufs=4))
        prod_p = ctx.enter_context(tc.tile_pool(name="prodp", bufs=3))
        wt_p = ctx.enter_context(tc.tile_pool(name="wt", bufs=2))
        wtv_p = ctx.enter_context(tc.tile_pool(name="wtv", bufs=1))
        pt_p = ctx.enter_context(tc.tile_pool(name="pt", bufs=4))
        rows_p = ctx.enter_context(tc.tile_pool(name="rows", bufs=3))
        bc_p = ctx.enter_context(tc.tile_pool(name="bc", bufs=2))
        sq_p = ctx.enter_context(tc.tile_pool(name="sqp", bufs=3))

        ps_tp = ctx.enter_context(
            tc.tile_pool(name="ps_tp", bufs=2, space="PSUM"))
        ps_mm = ctx.enter_context(
            tc.tile_pool(name="ps_mm", bufs=2, space="PSUM"))
        ps_row = ctx.enter_context(
            tc.tile_pool(name="ps_row", bufs=1, space="PSUM"))
        ps_o = ctx.enter_context(
            tc.tile_pool(name="ps_o", bufs=3, space="PSUM"))

        from contextlib import nullcontext
        loop_cm = tc.For_i(0, loop_iters, 1) if loop_iters else nullcontext()
        with loop_cm:
            _dmac = [0]

            def dma_rr(out, in_):
                i = _dmac[0]; _dmac[0] += 1
                eng = (nc.sync, nc.scalar)[i % 2]
                eng.dma_start(out=out, in_=in_)
            ident = consts.tile([128, 128], F32, tag="ident")
            nc.gpsimd.memset(ident, 0.0)
            nc.gpsimd.affine_select(
                out=ident, in_=ident, compare_op=OP.not_equal, fill=1.0,
                base=0, pattern=[[-1, 128]], channel_multiplier=1)
            # ================= standing tensors =================
            # F32R so LN stat matmuls can consume it directly (BIR requires
            # f32r matmul inputs to be produced as f32r, not bitcast).
            xres = sb.tile([128, CT, N], F32R, tag="xres")

            for ni in range(NT):
                natx = wnat_p.tile([128, C], F32, tag="wnat")
                dma_rr(natx, x_d[ni * 128:(ni + 1) * 128, :])
                for cp in range(CT // PK):
                    pst = ps_tp.tile([128, 128 * PK], F32, tag="tp")
                    for k in range(PK):
                        ci = cp * PK + k
                        nc.tensor.matmul(pst[:, k * 128:(k + 1) * 128],
                                         natx[:, ci * 128:(ci + 1) * 128], ident,
                                         is_transpose=True, start=True, stop=True)
                    nc.vector.tensor_copy(
                        xres[:, cp * PK:(cp + 1) * PK, ni * 128:(ni + 1) * 128],
                        pst.rearrange("p (a b) -> p a b", a=PK))

            def _stop(tag_):
                return stop_after is not None and stop_after == tag_

            # ================= constants =================
            ident_r = consts.tile([128, 128], F32R, tag="identr")
            nc.vector.tensor_copy(ident_r, ident)
            ident_b = consts.tile([128, 128], BF16, tag="identb")
            nc.vector.tensor_copy(ident_b, ident)
            ones65f = wnat_p.tile([65, 128], F32, tag="bnat")
            nc.vector.memset(ones65f, 1.0)
            ones65 = consts.tile([65, 128], F32R, tag="ones65")
            nc.vector.tensor_copy(ones65, ones65f)

            masks = consts.tile([128, 4, 512], BF16, tag="masks")
            nc.gpsimd.memset(masks, 1.0)
            for i in range(4):
                # keep where n >= m  <=>  s - r - delta >= 0 (delta = 128i)
                nc.gpsimd.affine_select(
                    out=masks[:, i, :], in_=masks[:, i, :], compare_op=OP.is_ge,
                    fill=0.0, base=-(128 * i), pattern=[[1, 512]],
                    channel_multiplier=-1)

            ones_invCf = consts.tile([128, 1], F32, tag="onescf")
            nc.gpsimd.memset(ones_invCf, 1.0 / C)
            ones_invC = consts.tile([128, 1], F32R, tag="onesc")
            nc.vector.tensor_copy(ones_invC, ones_invCf)
            eps_t = consts.tile([1, 1], F32, tag="eps")
            nc.vector.memset(eps_t, 1e-6)

            def bias_T(src, nch, tag):
                t = consts.tile([128, nch], F32, tag=tag)
                natb = wnat_p.tile([nch, 128], F32, tag="bnat")
                nc.sync.dma_start(out=natb,
                                  in_=src.rearrange("(a b) -> a b", b=128))
                pst = ps_tp.tile([128, 128 * PK], F32, tag="tp")
                nc.tensor.matmul(pst[:, 0:nch], natb, ident[0:nch, 0:nch],
                                 is_transpose=True, start=True, stop=True)
                nc.vector.tensor_copy(t, pst[:, 0:nch])
                return t

            qkvb_t = bias_T(qkvb_d.ap(), 3 * CT, "qkvbt")
            projb_t = bias_T(projb_d.ap(), CT, "projbt")
            fc1b_t = bias_T(fc1b_d.ap(), DT, "fc1bt")
            fc2b_t = bias_T(fc2b_d.ap(), CT, "fc2bt")
            adab_t = bias_T(adab_d.ap(), 6 * CT, "adabt")

            # ---- adaLN: silu(c) broadcast ----
            crow = wnat_p.tile([1, C], F32, tag="wnat")
            nc.sync.dma_start(out=crow, in_=c_d.ap().rearrange("(a c) -> a c", a=1))
            silu_row = wnat_p.tile([1, C], F32R, tag="wnat")
            nc.scalar.activation(silu_row, crow, AF.Sigmoid)
            nc.vector.tensor_mul(silu_row, silu_row, crow)
            silu_b = sb.tile([128, C], F32, tag="silub")
            for w0 in range(0, C, 512):
                w = min(512, C - w0)
                pb = ps_row.tile([128, 512], F32, tag="row")
                nc.tensor.matmul(pb[:, 0:w], ones65[0:1, :],
                                 silu_row[0:1, w0:w0 + w], start=True, stop=True)
                nc.vector.tensor_copy(silu_b[:, w0:w0 + w], pb[:, 0:w])

            modsb = consts.tile([128, 6 * CT], F32, tag="modsb")
            adadump = consts.tile([128, C], F32, tag="adadump")

            silu_bf = sb.tile([128, C], BF16, tag="silubf")
            nc.vector.tensor_copy(silu_bf, silu_b)

            def ada_block(jt):
                anat = ada_p.tile([128, C], F32, tag="ada")
                # SWDGE queue: keeps bulk ada traffic off the
                # latency-critical HWDGE weight rings
                nc.gpsimd.dma_start(out=anat,
                                    in_=adaw_d[jt * 128:(jt + 1) * 128, :])
                nc.vector.tensor_mul(anat, anat, silu_b)
                nc.scalar.activation(adadump, anat, AF.Identity,
                                     accum_out=modsb[:, jt:jt + 1])

            def ada_bias(lo, hi):
                nc.vector.tensor_add(modsb[:, lo:hi], modsb[:, lo:hi],
                                     adab_t[:, lo:hi])

            # shift/scale_msa now (LN1 path); the remaining 4 chunks are
            # interleaved into attention below so their ACT accumulates don't
            # head-of-line block attention's exps on the ACT queue.
            for jt in range(2 * CT):
                ada_block(jt)
            ada_bias(0, 2 * CT)
            sp_msa = consts.tile([128, CT], F32, tag="spmsa")
            nc.vector.tensor_scalar(sp_msa, modsb[:, CT:2 * CT], 1.0, None, OP.add)
            ada_todo = list(range(2 * CT, 6 * CT))

            def ada_step(n):
                for _ in range(min(n, len(ada_todo))):
                    ada_block(ada_todo.pop(0))

            # ================= helpers =================
            def layer_norm(dst, sh_off, sp_tile):
                for nj in range(NJ):
                    njs = slice(nj * 512, (nj + 1) * 512)
                    s_mu = ps_row.tile([1, 512], F32, tag="row")
                    for ci in range(CT):
                        nc.tensor.matmul(s_mu, ones_invC, xres[:, ci, njs],
                                         start=(ci == 0), stop=(ci == CT - 1))
                    s_sq = ps_o.tile([1, 512], F32, tag="po")
                    for ci in range(CT):
                        sqt = sq_p.tile([128, 512], F32R, tag="sq")
                        nc.scalar.activation(sqt, xres[:, ci, njs], AF.Square)
                        nc.tensor.matmul(s_sq, ones_invC, sqt,
                                         start=(ci == 0), stop=(ci == CT - 1))
                    t_mu = rows_p.tile([1, 512], F32, tag="rows")
                    nc.vector.tensor_copy(t_mu, s_mu)
                    t_var = rows_p.tile([1, 512], F32R, tag="rows")
                    nc.vector.tensor_mul(t_var, t_mu, t_mu)
                    nc.vector.tensor_sub(t_var, s_sq, t_var)
                    t_rstd = rows_p.tile([1, 512], F32R, tag="rows")
                    nc.scalar.activation(t_rstd, t_var, AF.Sqrt, bias=eps_t)
                    with nc.allow_low_precision(reason="f32r rstd"):
                        nc.vector.reciprocal(t_var, t_rstd)      # t_var = rstd
                    nc.vector.tensor_mul(t_rstd, t_mu, t_var)    # mu * rstd
                    bc_rp = ps_row.tile([128, 512], F32, tag="row")
                    nc.tensor.matmul(bc_rp, ones65[0:1, :], t_var[0:1, :],
                                     start=True, stop=True)
                    bc_r = bc_p.tile([128, 512], F32, tag="bc")
                    nc.vector.tensor_copy(bc_r, bc_rp)
                    bc_mrp = ps_row.tile([128, 512], F32, tag="row")
                    nc.tensor.matmul(bc_mrp, ones65[0:1, :], t_rstd[0:1, :],
                                     start=True, stop=True)
                    bc_mr = bc_p.tile([128, 512], F32, tag="bc")
                    nc.vector.tensor_copy(bc_mr, bc_mrp)
                    for ci in range(CT):
                        t = sq_p.tile([128, 512], F32, tag="sq")
                        nc.vector.tensor_mul(t, xres[:, ci, njs], bc_r)
                        nc.vector.tensor_sub(t, t, bc_mr)
                        nc.scalar.activation(
                            dst[:, ci, njs], t, AF.Identity,
                            scale=sp_tile[:, ci:ci + 1],
                            bias=modsb[:, sh_off + ci:sh_off + ci + 1])

            def transpose_rows(dst, src_dram, row0, nrows, hwdge=False):
                """dst[:, ci, rr*128 + f] = src[(row0+rr)*128 + f, ci*128 + p]

                Default path: SWDGE casts f32->bf16 in flight, PE transpose
                at 1 cyc/row, evac at 2x.  hwdge=True keeps the load on the
                HWDGE rings (f32) for latency-critical weights so they do
                not queue behind bulk SWDGE emissions on the Pool engine."""
                for rr_ in range(nrows):
                    natw = wnat_p.tile([128, C], F32R, tag="wnat")
                    dma_rr(natw, r(src_dram[(row0 + rr_) * 128:
                                           (row0 + rr_ + 1) * 128, :]))
                    pdt, idt = F32R, ident_r
                    for cp in range(CT // PK):
                        pst = ps_tp.tile([128, 128 * PK], pdt, tag="tp")
                        for k in range(PK):
                            ci = cp * PK + k
                            nc.tensor.matmul(pst[:, k * 128:(k + 1) * 128],
                                             natw[:, ci * 128:(ci + 1) * 128],
                                             idt, is_transpose=True,
                                             start=True, stop=True)
                        dsl = dst[:, cp * PK:(cp + 1) * PK,
                                  rr_ * 128:(rr_ + 1) * 128]
                        psr = pst.rearrange("p (a b) -> p a b", a=PK)
                        if (rr_ + cp) % 2 == 0:
                            nc.vector.tensor_copy(dsl, psr)
                        else:
                            nc.scalar.activation(dsl, psr, AF.Identity)

            # ================= attention =================
            if not _stop("x"):
              y = sb.tile([128, CT, N], BF16, tag="lnout")
              layer_norm(y, 0, sp_msa)
            if not _stop("x") and not _stop("ln1"):

              oT = sb.tile([128, CT, N], BF16, tag="oT")

              for g in range(NHG):
                  # ---- q,k (channel-major) ----
                  qkT = sb.tile([128, HG, N], BF16, tag="big")
                  for sl0 in range(0, HG, 2):
                      isq = sl0 < HG // 2
                      fi0 = (g * (HG // 2) + sl0) if isq else (
                          CT + g * (HG // 2) + (sl0 - HG // 2))
                      wtt = wt_p.tile([128, CT, 256], BF16, tag="wt")
                      transpose_rows(wtt, qkvw_d, fi0, 2, hwdge=True)
                      for sub in range(2):
                          sl = sl0 + sub
                          fi = fi0 + sub
                          for nj in range(NJ):
                              njs = slice(nj * 512, (nj + 1) * 512)
                              pmm = ps_mm.tile([128, 512], F32, tag="mm")
                              for ci in range(CT):
                                  nc.tensor.matmul(
                                      pmm,
                                      wtt[:, ci, sub * 128:(sub + 1) * 128],
                                      y[:, ci, njs],
                                      start=(ci == 0), stop=(ci == CT - 1))
                              nc.scalar.activation(qkT[:, sl, njs], pmm,
                                                   AF.Identity,
                                                   bias=qkvb_t[:, fi:fi + 1])

                  # ---- v (token-major, ones-augmented) ----
                  vaug = sb.tile([128, NT, HG, 65], BF16, tag="vaug")
                  nc.gpsimd.memset(vaug[:, :, :, 64:65], 1.0)
                  for vg in range((HG * 64 + 255) // 256):
                      wtt = wtv_p.tile([128, CT, 256], BF16, tag="wtv")
                      transpose_rows(wtt, qkvw_d, 2 * CT + g * (HG // 2) + vg * 2,
                                     2, hwdge=True)
                      vbrow = rows_p.tile([1, 512], F32R, tag="rows")
                      off = 2 * C + (g * HG + vg * 4) * 64
                      nc.sync.dma_start(
                          out=vbrow[0:1, 0:256],
                          in_=r(qkvb_d[off:off + 256]).rearrange("(a c) -> a c",
                                                                 a=1))
                      vbp = ps_row.tile([128, 512], F32, tag="row")
                      nc.tensor.matmul(vbp[:, 0:256], ones65[0:1, :],
                                       vbrow[0:1, 0:256], start=True, stop=True)
                      vb = bc_p.tile([128, 512], F32, tag="bc")
                      nc.vector.tensor_copy(vb[:, 0:256], vbp[:, 0:256])
                      for ni in range(NT):
                          pv = ps_mm.tile([128, 512], F32, tag="mm")
                          for ci in range(CT):
                              nc.tensor.matmul(
                                  pv[:, 0:256],
                                  y[:, ci, ni * 128:(ni + 1) * 128],
                                  wtt[:, ci, :],
                                  start=(ci == 0), stop=(ci == CT - 1))
                          nc.vector.tensor_add(
                              vaug[:, ni, vg * 4:vg * 4 + 4, 0:64],
                              pv[:, 0:256].rearrange("p (a b) -> p a b", a=4),
                              vb[:, 0:256].rearrange("p (a b) -> p a b", a=4))

                  # ---- attention proper (software-pipelined) ----
                  for nj in range(NJ):
                      njs = slice(nj * 512, (nj + 1) * 512)
                      mi_hi = min(NT, 4 * (nj + 1))
                      for hp in range(HG // 2):
                          ada_step(1)
                          qsl, ksl = hp, HG // 2 + hp
                          po0 = ps_o.tile([65, 512], F32, tag="po")
                          po1 = ps_o.tile([65, 512], F32, tag="po")
                          pos = [po0, po1]
                          pts_q = {}
                          for mi in range(mi_hi + 1):
                              if mi == mi_hi // 2:
                                  ada_step(1)
                              if mi < mi_hi:
                                  delta = 128 * mi - 512 * nj
                                  sps = []
                                  spool = ps_mm if mi % 2 == 0 else ps_tp
                                  for sub in range(2):
                                      base = sub * 64
                                      ps_s = spool.tile(
                                          [128, 512], F32,
                                          tag="mm" if spool is ps_mm else "tp",
                                          name="s%d" % sub)
                                      # pair shares PE via disjoint row groups
                                      nc.tensor.matmul(
                                          ps_s,
                                          qkT[base:base + 64, ksl,
                                              mi * 128:(mi + 1) * 128],
                                          qkT[base:base + 64, qsl, njs],
                                          start=True, stop=True)
                                      sps.append(ps_s)
                                  pts = []
                                  for sub in range(2):
                                      pt = pt_p.tile([128, 512], BF16,
                                                     tag="pt", name="pt%d" % sub)
                                      nc.scalar.activation(pt, sps[sub],
                                                           AF.Exp,
                                                           scale=D ** -0.5)
                                      if delta >= 0:
                                          # full-width mask mul (keeps the
                                          # zeroing off the Pool engine)
                                          nc.vector.tensor_mul(
                                              pt, pt, masks[:, delta // 128, :])
                                      pts.append(pt)
                                  pts_q[mi] = pts
                              if mi > 0:
                                  pprev = pts_q.pop(mi - 1)
                                  for sub in range(2):
                                      hl = 2 * hp + sub
                                      nc.tensor.matmul(pos[sub],
                                                       vaug[:, mi - 1, hl, :],
                                                       pprev[sub],
                                                       start=(mi - 1 == 0),
                                                       stop=(mi - 1 == mi_hi - 1))
                          for sub in range(2):
                              hl = 2 * hp + sub
                              h_glob = g * HG + hl
                              po = pos[sub]
                              srow = rows_p.tile([65, 512], F32R, tag="rows")
                              with nc.allow_low_precision(reason="f32r recip"):
                                  nc.vector.reciprocal(srow[64:65, :],
                                                       po[64:65, :])
                              rbp = ps_row.tile([128, 512], F32, tag="row")
                              nc.tensor.matmul(rbp[:, :], ones65[64:65, :],
                                               srow[64:65, :],
                                               start=True, stop=True)
                              rb = bc_p.tile([128, 512], F32, tag="bc")
                              nc.vector.tensor_copy(rb[0:64, :], rbp[0:64, :])
                              if sub == 0:
                                  nc.vector.tensor_mul(
                                      oT[0:64, h_glob // 2, njs],
                                      po[0:64, :], rb[0:64, :])
                              else:
                                  tsh = pt_p.tile([128, 512], BF16, tag="pt")
                                  nc.vector.tensor_mul(tsh[0:64, :],
                                                       po[0:64, :],
                                                       rb[0:64, :])
                                  nc.gpsimd.dma_start(
                                      out=oT[64:128, h_glob // 2, njs],
                                      in_=tsh[0:64, :])
              # drain any leftover ada blocks, then the gate/shift constants
              ada_step(len(ada_todo))
              ada_bias(2 * CT, 6 * CT)
              sp_mlp = consts.tile([128, CT], F32, tag="spmlp")
              nc.vector.tensor_scalar(sp_mlp, modsb[:, 4 * CT:5 * CT],
                                      1.0, None, OP.add)
              bg1 = consts.tile([128, CT], F32, tag="bg1")
              nc.vector.tensor_mul(bg1, modsb[:, 2 * CT:3 * CT], projb_t)
              bg2 = consts.tile([128, CT], F32, tag="bg2")
              nc.vector.tensor_mul(bg2, modsb[:, 5 * CT:6 * CT], fc2b_t)

            if stop_after is None:
              # ================= proj + gated residual =================
              for jc in range(CT):
                  wtt = wt_p.tile([128, CT, 128], BF16, tag="wt")
                  transpose_rows(wtt, projw_d, jc, 1)
                  for nj in range(NJ):
                      njs = slice(nj * 512, (nj + 1) * 512)
                      pmm = ps_mm.tile([128, 512], F32, tag="mm")
                      for ci in range(CT):
                          nc.tensor.matmul(pmm, wtt[:, ci, :], oT[:, ci, njs],
                                           start=(ci == 0), stop=(ci == CT - 1))
                      t = sq_p.tile([128, 512], F32, tag="sq")
                      nc.vector.tensor_scalar(
                          t, pmm, modsb[:, 2 * CT + jc:2 * CT + jc + 1],
                          bg1[:, jc:jc + 1], OP.mult, OP.add)
                      nc.vector.tensor_add(xres[:, jc, njs], xres[:, jc, njs], t)

              # ================= MLP =================
              z2 = sb.tile([128, CT, N], BF16, tag="lnout")
              layer_norm(z2, 3 * CT, sp_mlp)

              if NG > 1:
                  m2acc = sb.tile([128, CT, N], F32, tag="vaug")
              else:
                  m2acc = None

              for gi in range(NG):
                  h_t = sb.tile([128, G, NJ, 512], BF16, tag="big")
                  w2tg = sb.tile([128, G, CT, 128], BF16, tag="oT")
                  for dl0 in range(0, G, 2):
                    w1t = wt_p.tile([128, CT, 256], BF16, tag="wt")
                    transpose_rows(w1t, fc1w_d, gi * G + dl0, 2)
                    for dl in (dl0, dl0 + 1):
                      dd = gi * G + dl
                      dsub = dl - dl0
                      for nj in range(NJ):
                          njs = slice(nj * 512, (nj + 1) * 512)
                          ph = ps_mm.tile([128, 512], F32, tag="mm")
                          for ci in range(CT):
                              nc.tensor.matmul(
                                  ph, w1t[:, ci, dsub * 128:(dsub + 1) * 128],
                                  z2[:, ci, njs],
                                  start=(ci == 0), stop=(ci == CT - 1))
                          if not sim_safe:
                              nc.scalar.activation(h_t[:, dl, nj, :], ph,
                                                   AF.Gelu_apprx_tanh,
                                                   bias=fc1b_t[:, dd:dd + 1])
                          else:
                              # gelu_tanh decomposed for CoreSim (no Gelu impl)
                              s2pi = float(np.sqrt(2.0 / np.pi))
                              hs = sq_p.tile([128, 512], F32, tag="sq")
                              nc.scalar.activation(hs, ph, AF.Identity,
                                                   bias=fc1b_t[:, dd:dd + 1])
                              hq = sq_p.tile([128, 512], F32, tag="sq")
                              nc.scalar.activation(hq, ph, AF.Square,
                                                   bias=fc1b_t[:, dd:dd + 1])
                              nc.vector.tensor_scalar(hq, hq, s2pi * 0.044715,
                                                      s2pi, OP.mult, OP.add)
                              nc.vector.tensor_mul(hq, hq, hs)
                              nc.scalar.activation(hq, hq, AF.Tanh)
                              nc.vector.tensor_scalar(hq, hq, 0.5, 0.5,
                                                      OP.mult, OP.add)
                              nc.vector.tensor_mul(h_t[:, dl, nj, :], hq, hs)
                      natc = wnat_p.tile([128, C], F32R, tag="wnat")
                      dma_rr(natc.rearrange("p (a b) -> p a b", a=CT),
                             r(fc2w_d.ap().rearrange("(a p) d -> p a d", p=128)
                               [:, :, dd * 128:(dd + 1) * 128]))
                      for cp in range(CT // PK):
                          pst = ps_tp.tile([128, 128 * PK], F32R, tag="tp")
                          for k in range(PK):
                              jc = cp * PK + k
                              nc.tensor.matmul(
                                  pst[:, k * 128:(k + 1) * 128],
                                  natc[:, jc * 128:(jc + 1) * 128], ident_r,
                                  is_transpose=True, start=True, stop=True)
                          dsl = w2tg[:, dl, cp * PK:(cp + 1) * PK, :]
                          psr = pst.rearrange("p (a b) -> p a b", a=PK)
                          if (dl + cp) % 2 == 0:
                              nc.vector.tensor_copy(dsl, psr)
                          else:
                              nc.scalar.activation(dsl, psr, AF.Identity)
                  last = gi == NG - 1
                  for nj in range(NJ):
                      njs = slice(nj * 512, (nj + 1) * 512)
                      for jc in range(CT):
                          pm = ps_mm.tile([128, 512], F32, tag="mm")
                          for dl in range(G):
                              nc.tensor.matmul(pm, w2tg[:, dl, jc, :],
                                               h_t[:, dl, nj, :],
                                               start=(dl == 0), stop=(dl == G - 1))
                          if NG > 1 and gi == 0:
                              nc.vector.tensor_copy(m2acc[:, jc, njs], pm)
                          elif gi < NG - 1:
                              nc.vector.tensor_add(m2acc[:, jc, njs],
                                                   m2acc[:, jc, njs], pm)
                          else:
                              t = sq_p.tile([128, 512], F32, tag="sq")
                              if NG > 1:
                                  nc.vector.tensor_add(t, m2acc[:, jc, njs], pm)
                              else:
                                  nc.vector.tensor_copy(t, pm)
                              nc.vector.tensor_scalar(
                                  t, t, modsb[:, 5 * CT + jc:5 * CT + jc + 1],
                                  bg2[:, jc:jc + 1], OP.mult, OP.add)
                              nc.vector.tensor_add(xres[:, jc, njs],
                                                   xres[:, jc, njs], t)
                      if last:
                          # fused write-out for this nj's 4 token blocks
                          for ni in range(4 * nj, 4 * (nj + 1)):
                              onat = wnat_p.tile([128, C], F32, tag="wnat")
                              for cp in range(CT // PK):
                                  pst = ps_tp.tile([128, 128 * PK], F32R,
                                                   tag="tp")
                                  for k in range(PK):
                                      ci = cp * PK + k
                                      nc.tensor.matmul(
                                          pst[:, k * 128:(k + 1) * 128],
                                          xres[:, ci, ni * 128:(ni + 1) * 128],
                                          ident_r, is_transpose=True,
                                          start=True, stop=True)
                                  if (ni + cp) % 2 == 0:
                                      nc.vector.tensor_copy(
                                          onat[:, cp * 128 * PK:
                                               (cp + 1) * 128 * PK], pst)
                                  else:
                                      nc.scalar.activation(
                                          onat[:, cp * 128 * PK:
                                               (cp + 1) * 128 * PK], pst,
                                          AF.Identity)
                              nc.scalar.dma_start(
                                  out=out_d[ni * 128:(ni + 1) * 128, :],
                                  in_=onat)

            if stop_after is not None:
                # write out whatever xres holds so timing builds stay valid
                for ni in range(NT):
                    onat = wnat_p.tile([128, C], F32, tag="wnat")
                    for cp in range(CT // PK):
                        pst = ps_tp.tile([128, 128 * PK], F32R, tag="tp")
                        for k in range(PK):
                            ci = cp * PK + k
                            nc.tensor.matmul(pst[:, k * 128:(k + 1) * 128],
                                             xres[:, ci, ni * 128:(ni + 1) * 128],
                                             ident_r, is_transpose=True,
                                             start=True, stop=True)
                        nc.vector.tensor_copy(
                            onat[:, cp * 128 * PK:(cp + 1) * 128 * PK], pst)
                    nc.sync.dma_start(out=out_d[ni * 128:(ni + 1) * 128, :],
                                      in_=onat)

    if not nc.is_finalized():
        nc.finalize()
    return nc


_CACHE = {}


def get_program(key="full", **kw):
    if key not in _CACHE:
        _CACHE[key] = build_program(**kw)
    return _CACHE[key]


def make_in_maps(inputs):
    x = np.ascontiguousarray(np.asarray(inputs["x"], np.float32))
    c = np.ascontiguousarray(np.asarray(inputs["c"], np.float32))
    B = x.shape[0]
    shared = {k: np.ascontiguousarray(np.asarray(inputs[k], np.float32))
              for k in ("qkv_w", "qkv_b", "proj_w", "proj_b", "fc1_w",
                        "fc1_b", "fc2_w", "fc2_b", "ada_w", "ada_b")}
    return [dict(shared, x=x[b], cvec=c[b, 0]) for b in range(B)]


def kernel(**inputs):
    from concourse.bass_utils import run_bass_kernel_spmd

    x = np.asarray(inputs["x"])
    B, N, C = x.shape
    nc = get_program("full", N=N, C=C, H=16, DFF=4 * C, n_cores=B)
    in_maps = make_in_maps(inputs)
    res = run_bass_kernel_spmd(nc, in_maps, core_ids=list(range(B)))
    out = np.stack([res.results[b]["out"] for b in range(B)], axis=0)
    return out.astype(np.float32)


# revision 4
# speedup vs baseline: 1.0780x; 1.0503x over previous
"""DiT block kernel for Trainium2 (Bass/Tile), 8-core data-parallel, v2.

Changes vs v1 baseline:
  - bf16 weights + activations in all GEMM paths (fp32 residual + PSUM
    accumulation kept); enables FWL fast weight loads.
  - LN stats read xres via f32r bitcast (no ACT cast pass).
  - Attention inner loop software-pipelined: score matmul of mi+1 issues
    before the accumulate matmul of mi, hiding the exp latency; score
    tiles round-robin across two PSUM pools for 4 banks in flight.
  - dff_group=8: longer fc2 accumulation chains, fewer partial passes.
  - Write-out fused into the last MLP group (nj-outer loop).
  - DMAs issued from HWDGE engines only (sync/scalar).
"""

import sys

sys.path.insert(0, "/opt/trn_rl_repo")

import numpy as np

import concourse.bass as bass
import concourse.bacc as bacc
import concourse.mybir as mybir
from concourse import library_config
from concourse.tile import TileContext

F32 = mybir.dt.float32
F32R = mybir.dt.float32r
BF16 = mybir.dt.bfloat16
AF = mybir.ActivationFunctionType
OP = mybir.AluOpType


def r(ap):
    return ap.bitcast(F32R)


def build_program(N=1024, C=1024, H=16, DFF=4096, head_group=8, dff_group=8,
                  n_cores=8, sim_safe=False, stop_after=None, loop_iters=None):
    D = 64
    NT, CT, DT = N // 128, C // 128, DFF // 128
    NJ = N // 512
    HG = head_group
    NHG = H // HG
    G = dff_group
    NG = DT // G
    PK = min(4, CT)            # transpose pack (blocks per psum tile)
    assert H % HG == 0 and DT % G == 0 and N % 512 == 0 and CT % PK == 0
    assert HG % 2 == 0 and D == 64

    nc = bacc.Bacc("TRN2", target_bir_lowering=False, debug=False,
                   num_devices=n_cores, num_swdge_queues=4)

    x_d = nc.dram_tensor("x", [N, C], F32, kind="ExternalInput")
    c_d = nc.dram_tensor("cvec", [C], F32, kind="ExternalInput")
    qkvw_d = nc.dram_tensor("qkv_w", [3 * C, C], F32, kind="ExternalInput")
    qkvb_d = nc.dram_tensor("qkv_b", [3 * C], F32, kind="ExternalInput")
    projw_d = nc.dram_tensor("proj_w", [C, C], F32, kind="ExternalInput")
    projb_d = nc.dram_tensor("proj_b", [C], F32, kind="ExternalInput")
    fc1w_d = nc.dram_tensor("fc1_w", [DFF, C], F32, kind="ExternalInput")
    fc1b_d = nc.dram_tensor("fc1_b", [DFF], F32, kind="ExternalInput")
    fc2w_d = nc.dram_tensor("fc2_w", [C, DFF], F32, kind="ExternalInput")
    fc2b_d = nc.dram_tensor("fc2_b", [C], F32, kind="ExternalInput")
    adaw_d = nc.dram_tensor("ada_w", [6 * C, C], F32, kind="ExternalInput")
    adab_d = nc.dram_tensor("ada_b", [6 * C], F32, kind="ExternalInput")
    out_d = nc.dram_tensor("out", [N, C], F32, kind="ExternalOutput")

    from contextlib import ExitStack
    with TileContext(nc) as tc, ExitStack() as ctx:
        consts = ctx.enter_context(tc.tile_pool(name="consts", bufs=1))
        sb = ctx.enter_context(tc.tile_pool(name="sb", bufs=1))
        wnat_p = ctx.enter_context(tc.tile_pool(name="wnat", bufs=3))
        ada_p = ctx.enter_context(tc.tile_pool(name="adap", bufs=4))
        prod_p = ctx.enter_context(tc.tile_pool(name="prodp", bufs=3))
        wt_p = ctx.enter_context(tc.tile_pool(name="wt", bufs=2))
        wtv_p = ctx.enter_context(tc.tile_pool(name="wtv", bufs=1))
        pt_p = ctx.enter_context(tc.tile_pool(name="pt", bufs=4))
        rows_p = ctx.enter_context(tc.tile_pool(name="rows", bufs=3))
        bc_p = ctx.enter_context(tc.tile_pool(name="bc", bufs=2))
        sq_p = ctx.enter_context(tc.tile_pool(name="sqp", bufs=3))

        ps_tp = ctx.enter_context(
            tc.tile_pool(name="ps_tp", bufs=2, space="PSUM"))
        ps_mm = ctx.enter_context(
            tc.tile_pool(name="ps_mm", bufs=2, space="PSUM"))
        ps_row = ctx.enter_context(
            tc.tile_pool(name="ps_row", bufs=1, space="PSUM"))
        ps_o = ctx.enter_context(
            tc.tile_pool(name="ps_o", bufs=3, space="PSUM"))

        from contextlib import nullcontext
        loop_cm = tc.For_i(0, loop_iters, 1) if loop_iters else nullcontext()
        with loop_cm:
            _dmac = [0]

            def dma_rr(out, in_):
                i = _dmac[0]; _dmac[0] += 1
                eng = (nc.sync, nc.scalar)[i % 2]
                eng.dma_start(out=out, in_=in_)
            ident = consts.tile([128, 128], F32, tag="ident")
            nc.gpsimd.memset(ident, 0.0)
            nc.gpsimd.affine_select(
                out=ident, in_=ident, compare_op=OP.not_equal, fill=1.0,
                base=0, pattern=[[-1, 128]], channel_multiplier=1)
            # ================= standing tensors =================
            # F32R so LN stat matmuls can consume it directly (BIR requires
            # f32r matmul inputs to be produced as f32r, not bitcast).
            xres = sb.tile([128, CT, N], F32R, tag="xres")

            for ni in range(NT):
                natx = wnat_p.tile([128, C], F32, tag="wnat")
                if ni < 2:
                    # halves: first transposes start after 256KB, not 512KB
                    dma_rr(natx[:, 0:C // 2],
                           x_d[ni * 128:(ni + 1) * 128, 0:C // 2])
                    dma_rr(natx[:, C // 2:C],
                           x_d[ni * 128:(ni + 1) * 128, C // 2:C])
                else:
                    dma_rr(natx, x_d[ni * 128:(ni + 1) * 128, :])
                for cp in range(CT // PK):
                    pst = ps_tp.tile([128, 128 * PK], F32, tag="tp")
                    for k in range(PK):
                        ci = cp * PK + k
                        nc.tensor.matmul(pst[:, k * 128:(k + 1) * 128],
                                         natx[:, ci * 128:(ci + 1) * 128], ident,
                                         is_transpose=True, start=True, stop=True)
                    nc.vector.tensor_copy(
                        xres[:, cp * PK:(cp + 1) * PK, ni * 128:(ni + 1) * 128],
                        pst.rearrange("p (a b) -> p a b", a=PK))

            def _stop(tag_):
                return stop_after is not None and stop_after == tag_

            # ================= constants =================
            ident_r = consts.tile([128, 128], F32R, tag="identr")
            nc.vector.tensor_copy(ident_r, ident)
            ident_b = consts.tile([128, 128], BF16, tag="identb")
            nc.vector.tensor_copy(ident_b, ident)
            ones65f = wnat_p.tile([65, 128], F32, tag="bnat")
            nc.vector.memset(ones65f, 1.0)
            ones65 = consts.tile([65, 128], F32R, tag="ones65")
            nc.vector.tensor_copy(ones65, ones65f)

            masks = consts.tile([128, 4, 512], BF16, tag="masks")
            nc.gpsimd.memset(masks, 1.0)
            for i in range(4):
                # keep where n >= m  <=>  s - r - delta >= 0 (delta = 128i)
                nc.gpsimd.affine_select(
                    out=masks[:, i, :], in_=masks[:, i, :], compare_op=OP.is_ge,
                    fill=0.0, base=-(128 * i), pattern=[[1, 512]],
                    channel_multiplier=-1)

            ones_invCf = consts.tile([128, 1], F32, tag="onescf")
            nc.gpsimd.memset(ones_invCf, 1.0 / C)
            ones_invC = consts.tile([128, 1], F32R, tag="onesc")
            nc.vector.tensor_copy(ones_invC, ones_invCf)
            eps_t = consts.tile([1, 1], F32, tag="eps")
            nc.vector.memset(eps_t, 1e-6)

            def bias_T(src, nch, tag):
                t = consts.tile([128, nch], F32, tag=tag)
                natb = wnat_p.tile([nch, 128], F32, tag="bnat")
                nc.sync.dma_start(out=natb,
                                  in_=src.rearrange("(a b) -> a b", b=128))
                pst = ps_tp.tile([128, 128 * PK], F32, tag="tp")
                nc.tensor.matmul(pst[:, 0:nch], natb, ident[0:nch, 0:nch],
                                 is_transpose=True, start=True, stop=True)
                nc.vector.tensor_copy(t, pst[:, 0:nch])
                return t

            qkvb_t = bias_T(qkvb_d.ap(), 3 * CT, "qkvbt")
            projb_t = bias_T(projb_d.ap(), CT, "projbt")
            fc1b_t = bias_T(fc1b_d.ap(), DT, "fc1bt")
            fc2b_t = bias_T(fc2b_d.ap(), CT, "fc2bt")
            adab_t = bias_T(adab_d.ap(), 6 * CT, "adabt")

            # ---- adaLN: silu(c) broadcast ----
            crow = wnat_p.tile([1, C], F32, tag="wnat")
            nc.sync.dma_start(out=crow, in_=c_d.ap().rearrange("(a c) -> a c", a=1))
            silu_row = wnat_p.tile([1, C], F32R, tag="wnat")
            nc.scalar.activation(silu_row, crow, AF.Sigmoid)
            nc.vector.tensor_mul(silu_row, silu_row, crow)
            silu_b = sb.tile([128, C], F32, tag="silub")
            for w0 in range(0, C, 512):
                w = min(512, C - w0)
                pb = ps_row.tile([128, 512], F32, tag="row")
                nc.tensor.matmul(pb[:, 0:w], ones65[0:1, :],
                                 silu_row[0:1, w0:w0 + w], start=True, stop=True)
                nc.vector.tensor_copy(silu_b[:, w0:w0 + w], pb[:, 0:w])

            modsb = consts.tile([128, 6 * CT], F32, tag="modsb")
            adadump = consts.tile([128, C], F32, tag="adadump")

            silu_bf = sb.tile([128, C], BF16, tag="silubf")
            nc.vector.tensor_copy(silu_bf, silu_b)

            def ada_block(jt):
                anat = ada_p.tile([128, C], F32, tag="ada")
                # SWDGE queue: keeps bulk ada traffic off the
                # latency-critical HWDGE weight rings
                nc.gpsimd.dma_start(out=anat,
                                    in_=adaw_d[jt * 128:(jt + 1) * 128, :])
                nc.vector.tensor_mul(anat, anat, silu_b)
                nc.scalar.activation(adadump, anat, AF.Identity,
                                     accum_out=modsb[:, jt:jt + 1])

            def ada_bias(lo, hi):
                nc.vector.tensor_add(modsb[:, lo:hi], modsb[:, lo:hi],
                                     adab_t[:, lo:hi])

            # shift/scale_msa now (LN1 path); the remaining 4 chunks are
            # interleaved into attention below so their ACT accumulates don't
            # head-of-line block attention's exps on the ACT queue.
            for jt in range(2 * CT):
                ada_block(jt)
            ada_bias(0, 2 * CT)
            sp_msa = consts.tile([128, CT], F32, tag="spmsa")
            nc.vector.tensor_scalar(sp_msa, modsb[:, CT:2 * CT], 1.0, None, OP.add)
            ada_todo = list(range(2 * CT, 6 * CT))

            def ada_step(n):
                for _ in range(min(n, len(ada_todo))):
                    ada_block(ada_todo.pop(0))

            # ================= helpers =================
            def layer_norm(dst, sh_off, sp_tile):
                for nj in range(NJ):
                    njs = slice(nj * 512, (nj + 1) * 512)
                    s_mu = ps_row.tile([1, 512], F32, tag="row")
                    for ci in range(CT):
                        nc.tensor.matmul(s_mu, ones_invC, xres[:, ci, njs],
                                         start=(ci == 0), stop=(ci == CT - 1))
                    s_sq = ps_o.tile([1, 512], F32, tag="po")
                    for ci in range(CT):
                        sqt = sq_p.tile([128, 512], F32R, tag="sq")
                        nc.scalar.activation(sqt, xres[:, ci, njs], AF.Square)
                        nc.tensor.matmul(s_sq, ones_invC, sqt,
                                         start=(ci == 0), stop=(ci == CT - 1))
                    t_mu = rows_p.tile([1, 512], F32, tag="rows")
                    nc.vector.tensor_copy(t_mu, s_mu)
                    t_var = rows_p.tile([1, 512], F32R, tag="rows")
                    nc.vector.tensor_mul(t_var, t_mu, t_mu)
                    nc.vector.tensor_sub(t_var, s_sq, t_var)
                    t_rstd = rows_p.tile([1, 512], F32R, tag="rows")
                    nc.scalar.activation(t_rstd, t_var, AF.Sqrt, bias=eps_t)
                    with nc.allow_low_precision(reason="f32r rstd"):
                        nc.vector.reciprocal(t_var, t_rstd)      # t_var = rstd
                    nc.vector.tensor_mul(t_rstd, t_mu, t_var)    # mu * rstd
                    bc_rp = ps_row.tile([128, 512], F32, tag="row")
                    nc.tensor.matmul(bc_rp, ones65[0:1, :], t_var[0:1, :],
                                     start=True, stop=True)
                    bc_r = bc_p.tile([128, 512], F32, tag="bc")
                    nc.vector.tensor_copy(bc_r, bc_rp)
                    bc_mrp = ps_row.tile([128, 512], F32, tag="row")
                    nc.tensor.matmul(bc_mrp, ones65[0:1, :], t_rstd[0:1, :],
                                     start=True, stop=True)
                    bc_mr = bc_p.tile([128, 512], F32, tag="bc")
                    nc.vector.tensor_copy(bc_mr, bc_mrp)
                    for ci in range(CT):
                        t = sq_p.tile([128, 512], F32, tag="sq")
                        nc.vector.tensor_mul(t, xres[:, ci, njs], bc_r)
                        nc.vector.tensor_sub(t, t, bc_mr)
                        nc.scalar.activation(
                            dst[:, ci, njs], t, AF.Identity,
                            scale=sp_tile[:, ci:ci + 1],
                            bias=modsb[:, sh_off + ci:sh_off + ci + 1])

            def transpose_rows(dst, src_dram, row0, nrows, hwdge=False):
                """dst[:, ci, rr*128 + f] = src[(row0+rr)*128 + f, ci*128 + p]

                Default path: SWDGE casts f32->bf16 in flight, PE transpose
                at 1 cyc/row, evac at 2x.  hwdge=True keeps the load on the
                HWDGE rings (f32) for latency-critical weights so they do
                not queue behind bulk SWDGE emissions on the Pool engine."""
                for rr_ in range(nrows):
                    natw = wnat_p.tile([128, C], F32R, tag="wnat")
                    dma_rr(natw, r(src_dram[(row0 + rr_) * 128:
                                           (row0 + rr_ + 1) * 128, :]))
                    pdt, idt = F32R, ident_r
                    for cp in range(CT // PK):
                        pst = ps_tp.tile([128, 128 * PK], pdt, tag="tp")
                        for k in range(PK):
                            ci = cp * PK + k
                            nc.tensor.matmul(pst[:, k * 128:(k + 1) * 128],
                                             natw[:, ci * 128:(ci + 1) * 128],
                                             idt, is_transpose=True,
                                             start=True, stop=True)
                        dsl = dst[:, cp * PK:(cp + 1) * PK,
                                  rr_ * 128:(rr_ + 1) * 128]
                        psr = pst.rearrange("p (a b) -> p a b", a=PK)
                        if (rr_ + cp) % 2 == 0:
                            nc.vector.tensor_copy(dsl, psr)
                        else:
                            nc.scalar.activation(dsl, psr, AF.Identity)

            # ================= attention =================
            if not _stop("x"):
              y = sb.tile([128, CT, N], BF16, tag="lnout")
              layer_norm(y, 0, sp_msa)
            if not _stop("x") and not _stop("ln1"):

              oT = sb.tile([128, CT, N], BF16, tag="oT")

              for g in range(NHG):
                  # ---- q,k (channel-major) ----
                  qkT = sb.tile([128, HG, N], BF16, tag="big")
                  for sl0 in range(0, HG, 2):
                      isq = sl0 < HG // 2
                      fi0 = (g * (HG // 2) + sl0) if isq else (
                          CT + g * (HG // 2) + (sl0 - HG // 2))
                      wtt = wt_p.tile([128, CT, 256], BF16, tag="wt")
                      transpose_rows(wtt, qkvw_d, fi0, 2, hwdge=True)
                      for sub in range(2):
                          sl = sl0 + sub
                          fi = fi0 + sub
                          for nj in range(NJ):
                              njs = slice(nj * 512, (nj + 1) * 512)
                              pmm = ps_mm.tile([128, 512], F32, tag="mm")
                              for ci in range(CT):
                                  nc.tensor.matmul(
                                      pmm,
                                      wtt[:, ci, sub * 128:(sub + 1) * 128],
                                      y[:, ci, njs],
                                      start=(ci == 0), stop=(ci == CT - 1))
                              nc.scalar.activation(qkT[:, sl, njs], pmm,
                                                   AF.Identity,
                                                   bias=qkvb_t[:, fi:fi + 1])

                  # ---- v (token-major, ones-augmented) ----
                  vaug = sb.tile([128, NT, HG, 65], BF16, tag="vaug")
                  nc.gpsimd.memset(vaug[:, :, :, 64:65], 1.0)
                  for vg in range((HG * 64 + 255) // 256):
                      wtt = wtv_p.tile([128, CT, 256], BF16, tag="wtv")
                      transpose_rows(wtt, qkvw_d, 2 * CT + g * (HG // 2) + vg * 2,
                                     2, hwdge=True)
                      vbrow = rows_p.tile([1, 512], F32R, tag="rows")
                      off = 2 * C + (g * HG + vg * 4) * 64
                      nc.sync.dma_start(
                          out=vbrow[0:1, 0:256],
                          in_=r(qkvb_d[off:off + 256]).rearrange("(a c) -> a c",
                                                                 a=1))
                      vbp = ps_row.tile([128, 512], F32, tag="row")
                      nc.tensor.matmul(vbp[:, 0:256], ones65[0:1, :],
                                       vbrow[0:1, 0:256], start=True, stop=True)
                      vb = bc_p.tile([128, 512], F32, tag="bc")
                      nc.vector.tensor_copy(vb[:, 0:256], vbp[:, 0:256])
                      for ni in range(NT):
                          pv = ps_mm.tile([128, 512], F32, tag="mm")
                          for ci in range(CT):
                              nc.tensor.matmul(
                                  pv[:, 0:256],
                                  y[:, ci, ni * 128:(ni + 1) * 128],
                                  wtt[:, ci, :],
                                  start=(ci == 0), stop=(ci == CT - 1))
                          nc.vector.tensor_add(
                              vaug[:, ni, vg * 4:vg * 4 + 4, 0:64],
                              pv[:, 0:256].rearrange("p (a b) -> p a b", a=4),
                              vb[:, 0:256].rearrange("p (a b) -> p a b", a=4))

                  # ---- attention proper (software-pipelined) ----
                  for nj in range(NJ):
                      njs = slice(nj * 512, (nj + 1) * 512)
                      mi_hi = min(NT, 4 * (nj + 1))
                      for hp in range(HG // 2):
                          ada_step(1)
                          qsl, ksl = hp, HG // 2 + hp
                          po0 = ps_o.tile([65, 512], F32, tag="po")
                          po1 = ps_o.tile([65, 512], F32, tag="po")
                          pos = [po0, po1]
                          pts_q = {}
                          for mi in range(mi_hi + 1):
                              if mi == mi_hi // 2:
                                  ada_step(1)
                              if mi < mi_hi:
                                  delta = 128 * mi - 512 * nj
                                  sps = []
                                  spool = ps_mm if mi % 2 == 0 else ps_tp
                                  for sub in range(2):
                                      base = sub * 64
                                      ps_s = spool.tile(
                                          [128, 512], F32,
                                          tag="mm" if spool is ps_mm else "tp",
                                          name="s%d" % sub)
                                      # pair shares PE via disjoint row groups
                                      nc.tensor.matmul(
                                          ps_s,
                                          qkT[base:base + 64, ksl,
                                              mi * 128:(mi + 1) * 128],
                                          qkT[base:base + 64, qsl, njs],
                                          start=True, stop=True)
                                      sps.append(ps_s)
                                  pts = []
                                  for sub in range(2):
                                      pt = pt_p.tile([128, 512], BF16,
                                                     tag="pt", name="pt%d" % sub)
                                      nc.scalar.activation(pt, sps[sub],
                                                           AF.Exp,
                                                           scale=D ** -0.5)
                                      if delta >= 0:
                                          # full-width mask mul (keeps the
                                          # zeroing off the Pool engine)
                                          nc.vector.tensor_mul(
                                              pt, pt, masks[:, delta // 128, :])
                                      pts.append(pt)
                                  pts_q[mi] = pts
                              if mi > 0:
                                  pprev = pts_q.pop(mi - 1)
                                  for sub in range(2):
                                      hl = 2 * hp + sub
                                      nc.tensor.matmul(pos[sub],
                                                       vaug[:, mi - 1, hl, :],
                                                       pprev[sub],
                                                       start=(mi - 1 == 0),
                                                       stop=(mi - 1 == mi_hi - 1))
                          for sub in range(2):
                              hl = 2 * hp + sub
                              h_glob = g * HG + hl
                              po = pos[sub]
                              srow = rows_p.tile([65, 512], F32R, tag="rows")
                              with nc.allow_low_precision(reason="f32r recip"):
                                  nc.vector.reciprocal(srow[64:65, :],
                                                       po[64:65, :])
                              rbp = ps_row.tile([128, 512], F32, tag="row")
                              nc.tensor.matmul(rbp[:, :], ones65[64:65, :],
                                               srow[64:65, :],
                                               start=True, stop=True)
                              rb = bc_p.tile([128, 512], F32, tag="bc")
                              nc.vector.tensor_copy(rb[0:64, :], rbp[0:64, :])
                              if sub == 0:
                                  nc.vector.tensor_mul(
                                      oT[0:64, h_glob // 2, njs],
                                      po[0:64, :], rb[0:64, :])
                              else:
                                  tsh = pt_p.tile([128, 512], BF16, tag="pt")
                                  nc.vector.tensor_mul(tsh[0:64, :],
                                                       po[0:64, :],
                                                       rb[0:64, :])
                                  nc.gpsimd.dma_start(
                                      out=oT[64:128, h_glob // 2, njs],
                                      in_=tsh[0:64, :])
              # drain any leftover ada blocks, then the gate/shift constants
              ada_step(len(ada_todo))
              ada_bias(2 * CT, 6 * CT)
              sp_mlp = consts.tile([128, CT], F32, tag="spmlp")
              nc.vector.tensor_scalar(sp_mlp, modsb[:, 4 * CT:5 * CT],
                                      1.0, None, OP.add)
              bg1 = consts.tile([128, CT], F32, tag="bg1")
              nc.vector.tensor_mul(bg1, modsb[:, 2 * CT:3 * CT], projb_t)
              bg2 = consts.tile([128, CT], F32, tag="bg2")
              nc.vector.tensor_mul(bg2, modsb[:, 5 * CT:6 * CT], fc2b_t)

            if stop_after is None:
              # ================= proj + gated residual =================
              for jc in range(CT):
                  wtt = wt_p.tile([128, CT, 128], BF16, tag="wt")
                  transpose_rows(wtt, projw_d, jc, 1)
                  for nj in range(NJ):
                      njs = slice(nj * 512, (nj + 1) * 512)
                      pmm = ps_mm.tile([128, 512], F32, tag="mm")
                      for ci in range(CT):
                          nc.tensor.matmul(pmm, wtt[:, ci, :], oT[:, ci, njs],
                                           start=(ci == 0), stop=(ci == CT - 1))
                      t = sq_p.tile([128, 512], F32, tag="sq")
                      nc.vector.tensor_scalar(
                          t, pmm, modsb[:, 2 * CT + jc:2 * CT + jc + 1],
                          bg1[:, jc:jc + 1], OP.mult, OP.add)
                      nc.vector.tensor_add(xres[:, jc, njs], xres[:, jc, njs], t)

              # ================= MLP =================
              z2 = sb.tile([128, CT, N], BF16, tag="lnout")
              layer_norm(z2, 3 * CT, sp_mlp)

              if NG > 1:
                  m2acc = sb.tile([128, CT, N], F32, tag="vaug")
              else:
                  m2acc = None

              for gi in range(NG):
                  h_t = sb.tile([128, G, NJ, 512], BF16, tag="big")
                  w2tg = sb.tile([128, G, CT, 128], BF16, tag="oT")
                  for dl0 in range(0, G, 2):
                    w1t = wt_p.tile([128, CT, 256], BF16, tag="wt")
                    transpose_rows(w1t, fc1w_d, gi * G + dl0, 2)
                    for dl in (dl0, dl0 + 1):
                      dd = gi * G + dl
                      dsub = dl - dl0
                      for nj in range(NJ):
                          njs = slice(nj * 512, (nj + 1) * 512)
                          ph = ps_mm.tile([128, 512], F32, tag="mm")
                          for ci in range(CT):
                              nc.tensor.matmul(
                                  ph, w1t[:, ci, dsub * 128:(dsub + 1) * 128],
                                  z2[:, ci, njs],
                                  start=(ci == 0), stop=(ci == CT - 1))
                          if not sim_safe:
                              nc.scalar.activation(h_t[:, dl, nj, :], ph,
                                                   AF.Gelu_apprx_tanh,
                                                   bias=fc1b_t[:, dd:dd + 1])
                          else:
                              # gelu_tanh decomposed for CoreSim (no Gelu impl)
                              s2pi = float(np.sqrt(2.0 / np.pi))
                              hs = sq_p.tile([128, 512], F32, tag="sq")
                              nc.scalar.activation(hs, ph, AF.Identity,
                                                   bias=fc1b_t[:, dd:dd + 1])
                              hq = sq_p.tile([128, 512], F32, tag="sq")
                              nc.scalar.activation(hq, ph, AF.Square,
                                                   bias=fc1b_t[:, dd:dd + 1])
                              nc.vector.tensor_scalar(hq, hq, s2pi * 0.044715,
                                                      s2pi, OP.mult, OP.add)
                              nc.vector.tensor_mul(hq, hq, hs)
                              nc.scalar.activation(hq, hq, AF.Tanh)
                              nc.vector.tensor_scalar(hq, hq, 0.5, 0.5,
                                                      OP.mult, OP.add)
                              nc.vector.tensor_mul(h_t[:, dl, nj, :], hq, hs)
                      natc = wnat_p.tile([128, C], F32R, tag="wnat")
                      dma_rr(natc.rearrange("p (a b) -> p a b", a=CT),
                             r(fc2w_d.ap().rearrange("(a p) d -> p a d", p=128)
                               [:, :, dd * 128:(dd + 1) * 128]))
                      for cp in range(CT // PK):
                          pst = ps_tp.tile([128, 128 * PK], F32R, tag="tp")
                          for k in range(PK):
                              jc = cp * PK + k
                              nc.tensor.matmul(
                                  pst[:, k * 128:(k + 1) * 128],
                                  natc[:, jc * 128:(jc + 1) * 128], ident_r,
                                  is_transpose=True, start=True, stop=True)
                          dsl = w2tg[:, dl, cp * PK:(cp + 1) * PK, :]
                          psr = pst.rearrange("p (a b) -> p a b", a=PK)
                          if (dl + cp) % 2 == 0:
                              nc.vector.tensor_copy(dsl, psr)
                          else:
                              nc.scalar.activation(dsl, psr, AF.Identity)
                  last = gi == NG - 1
                  for nj in range(NJ):
                      njs = slice(nj * 512, (nj + 1) * 512)
                      for jc in range(CT):
                          pm = ps_mm.tile([128, 512], F32, tag="mm")
                          for dl in range(G):
                              nc.tensor.matmul(pm, w2tg[:, dl, jc, :],
                                               h_t[:, dl, nj, :],
                                               start=(dl == 0), stop=(dl == G - 1))
                          if NG > 1 and gi == 0:
                              nc.vector.tensor_copy(m2acc[:, jc, njs], pm)
                          elif gi < NG - 1:
                              nc.vector.tensor_add(m2acc[:, jc, njs],
                                                   m2acc[:, jc, njs], pm)
                          else:
                              t = sq_p.tile([128, 512], F32, tag="sq")
                              if NG > 1:
                                  nc.vector.tensor_add(t, m2acc[:, jc, njs], pm)
                              else:
                                  nc.vector.tensor_copy(t, pm)
                              nc.vector.tensor_scalar(
                                  t, t, modsb[:, 5 * CT + jc:5 * CT + jc + 1],
                                  bg2[:, jc:jc + 1], OP.mult, OP.add)
                              nc.vector.tensor_add(xres[:, jc, njs],
                                                   xres[:, jc, njs], t)
                      if last:
                          # fused write-out for this nj's 4 token blocks
                          for ni in range(4 * nj, 4 * (nj + 1)):
                              onat = wnat_p.tile([128, C], F32, tag="wnat")
                              for cp in range(CT // PK):
                                  pst = ps_tp.tile([128, 128 * PK], F32R,
                                                   tag="tp")
                                  for k in range(PK):
                                      ci = cp * PK + k
                                      nc.tensor.matmul(
                                          pst[:, k * 128:(k + 1) * 128],
                                          xres[:, ci, ni * 128:(ni + 1) * 128],
                                          ident_r, is_transpose=True,
                                          start=True, stop=True)
                                  if (ni + cp) % 2 == 0:
                                      nc.vector.tensor_copy(
                                          onat[:, cp * 128 * PK:
                                               (cp + 1) * 128 * PK], pst)
                                  else:
                                      nc.scalar.activation(
                                          onat[:, cp * 128 * PK:
                                               (cp + 1) * 128 * PK], pst,
                                          AF.Identity)
                              nc.scalar.dma_start(
                                  out=out_d[ni * 128:(ni + 1) * 128, :],
                                  in_=onat)

            if stop_after is not None:
                # write out whatever xres holds so timing builds stay valid
                for ni in range(NT):
                    onat = wnat_p.tile([128, C], F32, tag="wnat")
                    for cp in range(CT // PK):
                        pst = ps_tp.tile([128, 128 * PK], F32R, tag="tp")
                        for k in range(PK):
                            ci = cp * PK + k
                            nc.tensor.matmul(pst[:, k * 128:(k + 1) * 128],
                                             xres[:, ci, ni * 128:(ni + 1) * 128],
                                             ident_r, is_transpose=True,
                                             start=True, stop=True)
                        nc.vector.tensor_copy(
                            onat[:, cp * 128 * PK:(cp + 1) * 128 * PK], pst)
                    nc.sync.dma_start(out=out_d[ni * 128:(ni + 1) * 128, :],
                                      in_=onat)

    if not nc.is_finalized():
        nc.finalize()
    return nc


_CACHE = {}


def get_program(key="full", **kw):
    if key not in _CACHE:
        _CACHE[key] = build_program(**kw)
    return _CACHE[key]


def make_in_maps(inputs):
    x = np.ascontiguousarray(np.asarray(inputs["x"], np.float32))
    c = np.ascontiguousarray(np.asarray(inputs["c"], np.float32))
    B = x.shape[0]
    shared = {k: np.ascontiguousarray(np.asarray(inputs[k], np.float32))
              for k in ("qkv_w", "qkv_b", "proj_w", "proj_b", "fc1_w",
                        "fc1_b", "fc2_w", "fc2_b", "ada_w", "ada_b")}
    return [dict(shared, x=x[b], cvec=c[b, 0]) for b in range(B)]


def kernel(**inputs):
    from concourse.bass_utils import run_bass_kernel_spmd

    x = np.asarray(inputs["x"])
    B, N, C = x.shape
    nc = get_program("full", N=N, C=C, H=16, DFF=4 * C, n_cores=B)
    in_maps = make_in_maps(inputs)
    res = run_bass_kernel_spmd(nc, in_maps, core_ids=list(range(B)))
    out = np.stack([res.results[b]["out"] for b in range(B)], axis=0)
    return out.astype(np.float32)
